# revision 35
# baseline (speedup 1.0000x reference)
"""GPT-NeoX attention layer (B=2, S=2048, E=2048, H=16, partial RoPE 32/128)
as a Bass/Tile kernel for 8 Trainium2 NeuronCores.

Sharding: tensor-parallel across heads (2 heads per core, Megatron-style).
Each core computes QKV projection for its 2 heads over all tokens, applies
partial RoPE, runs causal attention, and produces a partial dense output
(contraction over its 256 columns of w_dense).  The 8 bf16 partial outputs
are summed on the host and the dense bias is added once on the host.

Everything on-device is bf16 (inputs pre-converted on the host); PSUM
accumulation stays fp32.  Key structure choices:

  qk_sb  [128, 4, SF]   Q^T/K^T per head (head dim on partitions) - scores
                        and y^T matmuls consume this directly.
  vnat   [128, SF/128, 256]  V in NATURAL [token, d] layout, produced in
                        phase 1 by x-stationary matmuls (x as lhsT), so no
                        PE transposes of V are ever needed.
  scores S^T = K^T.T @ Q^T in [sk, sq] blocks; exp on ScalarE (pipelined one
                        block behind the scores matmuls).
  softmax sums          via N=1 matmuls with the exp'd block as the
                        stationary operand (out [sq,1]): nearly free on PE,
                        instead of a 512-wide ones-matmul per block.
  normalize             reciprocal -> tiny PE transpose -> GPSIMD
                        partition_broadcast -> one DVE multiply per chunk.
  dense                 interleaved into later attention heads (fills the
                        tensor engine while ScalarE works through exp).
"""

import numpy as np
from contextlib import ExitStack

import concourse.bass as bass
import concourse.bacc as bacc
import concourse.mybir as mybir
import concourse.tile as tile
from concourse.masks import make_identity

AF = mybir.ActivationFunctionType
F32 = mybir.dt.float32
BF16 = mybir.dt.bfloat16

NEG_MASK = -1.0e9


class Cfg:
    def __init__(self, B=2, S=2048, E=2048, H=16, n_cores=8):
        self.B, self.S, self.E, self.H = B, S, E, H
        self.HS = 128                  # head size (one partition tile)
        self.ROT = 32                  # rotary dims
        self.n_cores = n_cores
        self.HPC = H // n_cores        # heads per core
        assert self.HPC == 2, "kernel assumes 2 heads per core"
        self.NQK = 2 * self.HPC        # q/k row tiles (h0q,h0k,h1q,h1k)
        self.VW = self.HPC * self.HS   # v natural width (d per core)
        self.RW = self.NQK * self.HS   # per-core q+k rows
        self.WCOLS = self.RW + self.VW
        self.SF = B * S
        self.KT = E // 128             # contraction tiles
        self.SC = 256                  # phase-1 token chunk
        self.NP1 = self.SF // self.SC
        self.G = self.SF // 4          # rope regroup width
        self.NQC = S // 512            # q chunks per (b, h)
        self.EO = E // 128             # dense output row tiles
        self.CT = self.HPC             # dense contraction tiles
        self.SCALE = 1.0 / np.sqrt(self.HS)
        assert S % 512 == 0 and E % 128 == 0 and self.SF % (4 * self.SC) == 0


class _Feeder:
    """Round-robin sink of deferred emission micro-steps (dense tiles)."""

    def __init__(self):
        self.gens = []

    def push(self, gen):
        self.gens.append(gen)

    def step(self):
        while self.gens:
            try:
                next(self.gens[0])
                return
            except StopIteration:
                self.gens.pop(0)

    def drain(self):
        while self.gens:
            g = self.gens.pop(0)
            for _ in g:
                pass


def build_program(cfg: Cfg, debug: bool = False) -> bass.Bass:
    B, S, E = cfg.B, cfg.S, cfg.E
    SF, KT, G = cfg.SF, cfg.KT, cfg.G
    SC, NQK, VW, RW = cfg.SC, cfg.NQK, cfg.VW, cfg.RW
    HPC, CT, EO = cfg.HPC, cfg.CT, cfg.EO
    NT = SF // 128                   # vnat token tiles

    nc = bacc.Bacc(None)
    xT = nc.dram_tensor("xT", [E, SF], BF16, kind="ExternalInput")
    wcat = nc.dram_tensor("wcat", [E, cfg.WCOLS], BF16, kind="ExternalInput")
    bqk = nc.dram_tensor("bqk", [RW], F32, kind="ExternalInput")
    vbbc = nc.dram_tensor("vbbc", [128, VW], F32, kind="ExternalInput")
    wdT = nc.dram_tensor("wdT", [VW, E], BF16, kind="ExternalInput")
    cosT = nc.dram_tensor("cosT", [32, SF], BF16, kind="ExternalInput")
    sinT = nc.dram_tensor("sinT", [32, SF], BF16, kind="ExternalInput")
    maskT = nc.dram_tensor("maskT", [128, 128], BF16, kind="ExternalInput")
    outT = nc.dram_tensor("outT", [E, SF], BF16, kind="ExternalOutput")

    with tile.TileContext(nc) as tc, ExitStack() as stk:
        consts = stk.enter_context(tc.tile_pool(name="consts", bufs=1))
        bigp = stk.enter_context(tc.tile_pool(name="big", bufs=1))
        qk_sb = bigp.tile([128, NQK, SF], BF16)
        vnat = bigp.tile([128, NT, VW], BF16)
        yT_sb = bigp.tile([128, CT, SF], BF16)

        # RoPE: rotate_half is a partition swap within the 32 rot rows ->
        # DVE stream_shuffle (per-quadrant permutation) + elementwise
        # combine in [32, cols] layout, zero DMAs.  Each 1024-col slice is
        # emitted as soon as the phase-1 chunks covering it are done.
        SW = 1024
        rope_mask = [(i + 16) % 32 for i in range(32)]
        ropep = stk.enter_context(tc.tile_pool(name="rope", bufs=2))

        def rope_slice(i, sl):
            cs = slice(sl * SW, (sl + 1) * SW)
            blk = qk_sb[0:cfg.ROT, i, cs]
            sw = ropep.tile([32, SW], BF16, tag="swap", name="sw")
            nc.vector.stream_shuffle(sw, blk, rope_mask)
            with nc.allow_low_precision(reason="bf16 rope"):
                nc.vector.tensor_mul(sw, sw, sin_sb[:, cs])
                nc.vector.tensor_mul(blk, blk, cos_sb[:, cs])
                nc.vector.tensor_add(blk, blk, sw)

        # ---------------- Phase 1: QKV projection ------------------------
        with tc.tile_pool(name="wq", bufs=1) as wp, \
             tc.tile_pool(name="xs", bufs=2) as xp, \
             tc.tile_pool(name="ps1", bufs=2, space="PSUM") as pp1:
            w_sb = wp.tile([128, KT, cfg.WCOLS], BF16)
            w_view = wcat.rearrange("(kt p) r -> p kt r", p=128)
            x_view = xT.rearrange("(kt p) s -> p kt s", p=128)

            # interleave per-kt w loads with quarters of the first x chunk
            xt0 = xp.tile([128, KT, SC], BF16, tag="xt")
            ktg = max(1, KT // 4)
            for q0 in range(0, KT, ktg):
                q1 = min(q0 + ktg, KT)
                for kt in range(q0, q1):
                    nc.sync.dma_start(out=w_sb[:, kt, :], in_=w_view[:, kt, :])
                nc.sync.dma_start(out=xt0[:, q0:q1, :],
                                  in_=x_view[:, q0:q1, 0:SC])
            xt1 = xp.tile([128, KT, SC], BF16, tag="xt")
            nc.sync.dma_start(out=xt1[:, :, :], in_=x_view[:, :, SC:2 * SC])

            # constants (after the critical w/x stream)
            ident = consts.tile([128, 128], F32)
            make_identity(nc, ident)
            identB = consts.tile([128, 128], BF16)
            with nc.allow_low_precision(reason="bf16 identity"):
                nc.vector.tensor_copy(identB, ident)
            ones_tmp = consts.tile([128, 1], BF16, tag="onestmp")
            nc.vector.memset(ones_tmp, 1.0)
            ones_col = ones_tmp
            mask_sb = consts.tile([128, 128], BF16)
            nc.sync.dma_start(out=mask_sb, in_=maskT[:, :])
            bqk_sb = consts.tile([128, NQK], F32)
            nc.sync.dma_start(out=bqk_sb,
                              in_=bqk.rearrange("(rt p) -> p rt", p=128))
            vb_sb = consts.tile([128, VW], F32)
            nc.sync.dma_start(out=vb_sb, in_=vbbc[:, :])
            cos_sb = consts.tile([32, SF], BF16, tag="costab")
            sin_sb = consts.tile([32, SF], BF16, tag="sintab")
            nc.sync.dma_start(out=cos_sb, in_=cosT[:, :])
            nc.sync.dma_start(out=sin_sb, in_=sinT[:, :])
            wd_sb = consts.tile([128, CT, E], BF16, tag="wd")
            nc.sync.dma_start(
                out=wd_sb[:, :, :],
                in_=wdT.rearrange("(ct p) e -> p ct e", p=128))

            ntile = SC // 128   # v token sub-tiles per chunk (=2)
            for sc in range(cfg.NP1):
                if sc == 0:
                    xt = xt0
                elif sc == 1:
                    xt = xt1
                else:
                    xt = xp.tile([128, KT, SC], BF16, tag="xt")
                    nc.sync.dma_start(
                        out=xt[:, :, :],
                        in_=x_view[:, :, sc * SC:(sc + 1) * SC])
                qkps = [pp1.tile([128, 512], F32, tag=f"qk{i}", name=f"qkps{i}")
                        for i in range(NQK // 2)]
                vps = pp1.tile([128, 512], F32, tag="v")
                # NOTE: a start=True matmul marks the PSUM bank's whole 2KB
                # zero-region pending-zero, so only the FIRST matmul into
                # each bank starts; co-resident column groups accumulate
                # onto pending-zero bytes.
                for kt in range(KT):
                    fl, ll = (kt == 0), (kt == KT - 1)
                    for i in range(NQK):
                        nc.tensor.matmul(
                            qkps[i // 2][:, 256 * (i % 2):256 * (i % 2) + SC],
                            w_sb[:, kt, 128 * i:128 * (i + 1)],
                            xt[:, kt, :],
                            start=fl and i % 2 == 0,
                            stop=ll and i % 2 == 1, skip_group_check=True)
                    for t in range(ntile):
                        nc.tensor.matmul(
                            vps[:, VW * t:VW * (t + 1)],
                            xt[:, kt, 128 * t:128 * (t + 1)],
                            w_sb[:, kt, RW:RW + VW],
                            start=fl and t == 0,
                            stop=ll and t == ntile - 1, skip_group_check=True)
                for i in range(NQK):
                    nc.scalar.activation(
                        qk_sb[:, i, sc * SC:(sc + 1) * SC],
                        qkps[i // 2][:, 256 * (i % 2):256 * (i % 2) + SC],
                        AF.Identity, bias=bqk_sb[:, i:i + 1])
                with nc.allow_low_precision(reason="bf16 v eviction"):
                    for t in range(ntile):
                        nc.vector.tensor_add(
                            vnat[:, sc * ntile + t, :],
                            vps[:, VW * t:VW * (t + 1)], vb_sb)
                if (sc + 1) % (SW // SC) == 0:
                    for i in range(NQK):
                        rope_slice(i, sc // (SW // SC))

        # ---------------- Attention + interleaved dense -------------------
        feeder = _Feeder()
        with tc.tile_pool(name="pstrip", bufs=3) as ptp, \
             tc.tile_pool(name="norm", bufs=2) as npool, \
             tc.tile_pool(name="outsb", bufs=4) as op, \
             tc.tile_pool(name="psA", bufs=3, space="PSUM") as psA, \
             tc.tile_pool(name="psY", bufs=2, space="PSUM") as psY, \
             tc.tile_pool(name="psS", bufs=1, space="PSUM") as psS, \
             tc.tile_pool(name="psD", bufs=2, space="PSUM") as psD:

            def dense_steps(b, scp):
                # one (eo) output row-tile over two 512-token col chunks
                for eo in range(EO):
                    ot = op.tile([128, 1024], BF16, tag="out")
                    for t in range(2):
                        col = b * S + (2 * scp + t) * 512
                        pd = psD.tile([128, 512], F32, tag="D")
                        for ct in range(CT):
                            nc.tensor.matmul(
                                pd,
                                wd_sb[:, ct, 128 * eo:128 * (eo + 1)],
                                yT_sb[:, ct, col:col + 512],
                                start=(ct == 0), stop=(ct == CT - 1),
                                skip_group_check=True)
                        yield
                        # evictions stay off ScalarE: exp saturates it
                        with nc.allow_low_precision(reason="bf16 out"):
                            nc.vector.tensor_copy(
                                ot[:, 512 * t:512 * (t + 1)], pd)
                    nc.sync.dma_start(
                        out=outT[128 * eo:128 * (eo + 1),
                                 b * S + scp * 1024:b * S + (scp + 1) * 1024],
                        in_=ot)
                    yield

            def attention(b, hl, on_chain=None):
                scol = b * S
                q_t = qk_sb[:, 2 * hl, scol:scol + S]
                k_t = qk_sb[:, 2 * hl + 1, scol:scol + S]
                pending = [None]   # chunk-end normalization closure

                def emit_chain(c, psYt, psSt):
                    recip = npool.tile([128, 4], BF16, tag="recip")
                    with nc.allow_low_precision(reason="bf16 recip"):
                        nc.vector.reciprocal(recip, psSt[:, 0:4])
                    # transpose each recip column to partition 0 ([1, 128])
                    rps = psA.tile([128, 512], BF16, tag="A", name="rps")
                    for g in range(4):
                        nc.tensor.matmul(
                            rps[0:1, 128 * g:128 * (g + 1)],
                            recip[:, g:g + 1], identB,
                            is_transpose=True, start=(g == 0), stop=(g == 3),
                            skip_group_check=True)
                    rT = npool.tile([1, 512], F32, tag="rT")
                    nc.vector.tensor_copy(rT, rps[0:1, 0:512])
                    bc = npool.tile([128, 512], F32, tag="bc")
                    for g in range(4):
                        nc.gpsimd.partition_broadcast(
                            bc[:, 128 * g:128 * (g + 1)],
                            rT[0:1, 128 * g:128 * (g + 1)])
                    with nc.allow_low_precision(reason="bf16 y eviction"):
                        nc.vector.tensor_mul(
                            yT_sb[:, hl, scol + c * 512:scol + (c + 1) * 512],
                            psYt[:, 0:512], bc)
                    if on_chain is not None:
                        on_chain(c)

                for c in range(cfg.NQC):
                    nj = 4 * (c + 1)
                    psYt = psY.tile([128, 512], F32, tag="Y")
                    psSt = psS.tile([128, 4], F32, tag="S")
                    prev = None

                    def emit_ys(j, pT, off, g0, psYt=psYt, psSt=psSt, c=c,
                                nj=nj):
                        nc.tensor.matmul(
                            psYt[:, off:512],
                            vnat[:, b * (S // 128) + j, 128 * hl:128 * (hl + 1)],
                            pT[:, off:512],
                            start=(j == 0), stop=(j == nj - 1),
                            skip_group_check=True)
                        for g in range(g0, 4):
                            # start only on the very first sums matmul of the
                            # chunk (bank-wide zero region); later columns
                            # accumulate onto pending-zero bytes.
                            nc.tensor.matmul(
                                psSt[:, g:g + 1],
                                pT[:, 128 * g:128 * (g + 1)], ones_col,
                                start=(j == 0 and g == 0),
                                stop=(j == nj - 1 and g == 3),
                                skip_group_check=True)

                    for j in range(nj):
                        g0 = max(0, j - 4 * c)
                        off = 128 * g0
                        ps = psA.tile([128, 512], F32, tag="A")
                        nc.tensor.matmul(
                            ps[:, off:512],
                            k_t[:, 128 * j:128 * (j + 1)],
                            q_t[:, c * 512 + off:(c + 1) * 512],
                            start=True, stop=True, skip_group_check=True)
                        pT = ptp.tile([128, 512], BF16, tag="p")
                        nc.scalar.activation(
                            pT[:, off:512], ps[:, off:512], AF.Exp,
                            scale=cfg.SCALE)
                        if j >= 4 * c:
                            # causal mask as a cheap post-exp 0/1 multiply
                            with nc.allow_low_precision(reason="bf16 mask"):
                                nc.vector.tensor_mul(
                                    pT[:, off:off + 128],
                                    pT[:, off:off + 128], mask_sb)
                        if prev is not None:
                            emit_ys(*prev)
                        if j == 0 and pending[0] is not None:
                            pending[0]()
                            pending[0] = None
                        feeder.step()
                        feeder.step()
                        prev = (j, pT, off, g0)
                    emit_ys(*prev)
                    feeder.step()
                    pending[0] = (lambda c=c, y=psYt, s=psSt:
                                  emit_chain(c, y, s))
                if pending[0] is not None:
                    pending[0]()
                    pending[0] = None

            for b in range(B):
                for hl in range(HPC):
                    if hl == HPC - 1:
                        # dense cols [0:1024*(scp+1)) ready once this head's
                        # chunk 2*scp+1 is normalized
                        hook = (lambda c, b=b: feeder.push(
                            dense_steps(b, (c - 1) // 2)) if c % 2 == 1
                            else None)
                    else:
                        hook = None
                    attention(b, hl, on_chain=hook)
            feeder.drain()

            if debug:
                dqk = nc.dram_tensor("dbg_qk", [128, NQK, SF], BF16,
                                     kind="ExternalOutput")
                dv = nc.dram_tensor("dbg_v", [128, NT, VW], BF16,
                                    kind="ExternalOutput")
                dy = nc.dram_tensor("dbg_y", [128, CT, SF], BF16,
                                    kind="ExternalOutput")
                nc.sync.dma_start(out=dqk[:, :, :], in_=qk_sb[:, :, :])
                nc.sync.dma_start(out=dv[:, :, :], in_=vnat[:, :, :])
                nc.sync.dma_start(out=dy[:, :, :], in_=yT_sb[:, :, :])

    nc.finalize()
    return nc


# ---------------------------------------------------------------------------
# Host-side input preparation / sharding
# ---------------------------------------------------------------------------

def _bf16(a):
    import ml_dtypes
    return np.ascontiguousarray(a, np.float32).astype(ml_dtypes.bfloat16)


def _rope_tables(cfg: Cfg):
    inv_freq = 1.0 / (10000.0 ** (np.arange(0, cfg.ROT, 2, dtype=np.float64)
                                  / cfg.ROT))
    t = np.arange(cfg.S, dtype=np.float64)
    freqs = np.outer(t, inv_freq)                       # [S, 16]
    emb = np.concatenate([freqs, freqs], axis=-1)       # [S, 32]
    cos = np.cos(emb).T.astype(np.float32)              # [32, S]
    sin = np.sin(emb).T.astype(np.float32)
    cosF = np.tile(cos, (1, cfg.B))                     # [32, SF]
    sinF = np.tile(sin, (1, cfg.B))
    sinF[:cfg.ROT // 2] *= -1.0                         # fold rotate_half sign
    return _bf16(cosF), _bf16(sinF)


def make_in_maps(cfg: Cfg, x, w_qkv, b_qkv, w_dense):
    HS, HPC = cfg.HS, cfg.HPC
    xTb = _bf16(np.ascontiguousarray(
        np.asarray(x, np.float32).reshape(cfg.SF, cfg.E).T))
    cos128, sin128s = _rope_tables(cfg)
    p = np.arange(128)[:, None]
    f = np.arange(128)[None, :]
    maskT = _bf16(np.where(p <= f, 1.0, 0.0))   # post-exp 0/1 causal mask
    in_maps = []
    for i in range(cfg.n_cores):
        heads = [HPC * i + h for h in range(HPC)]
        qk_rows = np.concatenate(
            [np.arange(h * 3 * HS + qk * HS, h * 3 * HS + (qk + 1) * HS)
             for h in heads for qk in range(2)])
        v_rows = np.concatenate(
            [np.arange(h * 3 * HS + 2 * HS, h * 3 * HS + 3 * HS)
             for h in heads])
        wcat = np.concatenate(
            [np.asarray(w_qkv, np.float32)[qk_rows, :].T,
             np.asarray(w_qkv, np.float32)[v_rows, :].T], axis=1)
        cols = slice(i * cfg.VW, (i + 1) * cfg.VW)
        in_maps.append({
            "xT": xTb,
            "wcat": _bf16(wcat),
            "bqk": np.ascontiguousarray(
                np.asarray(b_qkv, np.float32)[qk_rows]),
            "vbbc": np.ascontiguousarray(np.tile(
                np.asarray(b_qkv, np.float32)[v_rows][None, :], (128, 1))),
            "wdT": _bf16(np.asarray(w_dense, np.float32)[:, cols].T),
            "cosT": cos128,
            "sinT": sin128s,
            "maskT": maskT,
        })
    return in_maps


def combine_outputs(cfg: Cfg, results, b_dense):
    acc = np.zeros((cfg.E, cfg.SF), dtype=np.float64)
    for r in results:
        acc += np.asarray(r["outT"]).astype(np.float64)
    out = acc.T.reshape(cfg.B, cfg.S, cfg.E) + \
        np.asarray(b_dense, np.float64)
    return out.astype(np.float32)


_PROGRAM_CACHE = {}


def kernel(x, w_qkv, b_qkv, w_dense, b_dense):
    from concourse.bass_utils import run_bass_kernel_spmd

    cfg = Cfg()
    key = "full"
    if key not in _PROGRAM_CACHE:
        _PROGRAM_CACHE[key] = build_program(cfg)
    nc = _PROGRAM_CACHE[key]
    in_maps = make_in_maps(cfg, np.asarray(x), np.asarray(w_qkv),
                           np.asarray(b_qkv), np.asarray(w_dense))
    res = run_bass_kernel_spmd(nc, in_maps, list(range(cfg.n_cores)))
    return combine_outputs(cfg, res.results, np.asarray(b_dense))


# revision 36
# speedup vs baseline: 1.0279x; 1.0279x over previous
"""GPT-NeoX attention layer (B=2, S=2048, E=2048, H=16, partial RoPE 32/128)
as a Bass/Tile kernel for 8 Trainium2 NeuronCores.

Sharding: tensor-parallel across heads (2 heads per core, Megatron-style).
Each core computes QKV projection for its 2 heads over all tokens, applies
partial RoPE, runs causal attention, and produces a partial dense output
(contraction over its 256 columns of w_dense).  The 8 bf16 partial outputs
are summed on the host and the dense bias is added once on the host.

Everything on-device is bf16 (inputs pre-converted on the host); PSUM
accumulation stays fp32.  Key structure choices:

  qk_sb  [128, 4, SF]   Q^T/K^T per head (head dim on partitions) - scores
                        and y^T matmuls consume this directly.
  vnat   [128, SF/128, 256]  V in NATURAL [token, d] layout, produced in
                        phase 1 by x-stationary matmuls (x as lhsT), so no
                        PE transposes of V are ever needed.
  scores S^T = K^T.T @ Q^T in [sk, sq] blocks; exp on ScalarE (pipelined one
                        block behind the scores matmuls).
  softmax sums          via N=1 matmuls with the exp'd block as the
                        stationary operand (out [sq,1]): nearly free on PE,
                        instead of a 512-wide ones-matmul per block.
  normalize             reciprocal -> tiny PE transpose -> GPSIMD
                        partition_broadcast -> one DVE multiply per chunk.
  dense                 interleaved into later attention heads (fills the
                        tensor engine while ScalarE works through exp).
"""

import numpy as np
from contextlib import ExitStack

import concourse.bass as bass
import concourse.bacc as bacc
import concourse.mybir as mybir
import concourse.tile as tile
from concourse.masks import make_identity

AF = mybir.ActivationFunctionType
F32 = mybir.dt.float32
BF16 = mybir.dt.bfloat16

NEG_MASK = -1.0e9


class Cfg:
    def __init__(self, B=2, S=2048, E=2048, H=16, n_cores=8):
        self.B, self.S, self.E, self.H = B, S, E, H
        self.HS = 128                  # head size (one partition tile)
        self.ROT = 32                  # rotary dims
        self.n_cores = n_cores
        self.HPC = H // n_cores        # heads per core
        assert self.HPC == 2, "kernel assumes 2 heads per core"
        self.NQK = 2 * self.HPC        # q/k row tiles (h0q,h0k,h1q,h1k)
        self.VW = self.HPC * self.HS   # v natural width (d per core)
        self.RW = self.NQK * self.HS   # per-core q+k rows
        self.WCOLS = self.RW + self.VW
        self.SF = B * S
        self.KT = E // 128             # contraction tiles
        self.SC = 256                  # phase-1 token chunk
        self.NP1 = self.SF // self.SC
        self.G = self.SF // 4          # rope regroup width
        self.NQC = S // 512            # q chunks per (b, h)
        self.EO = E // 128             # dense output row tiles
        self.CT = self.HPC             # dense contraction tiles
        self.SCALE = 1.0 / np.sqrt(self.HS)
        assert S % 512 == 0 and E % 128 == 0 and self.SF % (4 * self.SC) == 0


class _Feeder:
    """Round-robin sink of deferred emission micro-steps (dense tiles)."""

    def __init__(self):
        self.gens = []

    def push(self, gen):
        self.gens.append(gen)

    def step(self):
        while self.gens:
            try:
                next(self.gens[0])
                return
            except StopIteration:
                self.gens.pop(0)

    def drain(self):
        while self.gens:
            g = self.gens.pop(0)
            for _ in g:
                pass


def build_program(cfg: Cfg, debug: bool = False) -> bass.Bass:
    B, S, E = cfg.B, cfg.S, cfg.E
    SF, KT, G = cfg.SF, cfg.KT, cfg.G
    SC, NQK, VW, RW = cfg.SC, cfg.NQK, cfg.VW, cfg.RW
    HPC, CT, EO = cfg.HPC, cfg.CT, cfg.EO
    NT = SF // 128                   # vnat token tiles

    nc = bacc.Bacc(None)
    xT = nc.dram_tensor("xT", [E, SF], BF16, kind="ExternalInput")
    wcat = nc.dram_tensor("wcat", [E, cfg.WCOLS], BF16, kind="ExternalInput")
    bqk = nc.dram_tensor("bqk", [RW], F32, kind="ExternalInput")
    vbbc = nc.dram_tensor("vbbc", [128, VW], F32, kind="ExternalInput")
    wdT = nc.dram_tensor("wdT", [VW, E], BF16, kind="ExternalInput")
    cosT = nc.dram_tensor("cosT", [32, SF], BF16, kind="ExternalInput")
    sinT = nc.dram_tensor("sinT", [32, SF], BF16, kind="ExternalInput")
    maskT = nc.dram_tensor("maskT", [128, 128], BF16, kind="ExternalInput")
    outT = nc.dram_tensor("outT", [E, SF], BF16, kind="ExternalOutput")

    with tile.TileContext(nc) as tc, ExitStack() as stk:
        consts = stk.enter_context(tc.tile_pool(name="consts", bufs=1))
        bigp = stk.enter_context(tc.tile_pool(name="big", bufs=1))
        qk_sb = bigp.tile([128, NQK, SF], BF16)
        vnat = bigp.tile([128, NT, VW], BF16)
        yT_sb = bigp.tile([128, CT, SF], BF16)

        # RoPE: rotate_half is a partition swap within the 32 rot rows ->
        # DVE stream_shuffle (per-quadrant permutation) + elementwise
        # combine in [32, cols] layout, zero DMAs.  Each 1024-col slice is
        # emitted as soon as the phase-1 chunks covering it are done.
        SW = 1024
        rope_mask = [(i + 16) % 32 for i in range(32)]
        ropep = stk.enter_context(tc.tile_pool(name="rope", bufs=2))

        def rope_slice(i, sl):
            cs = slice(sl * SW, (sl + 1) * SW)
            blk = qk_sb[0:cfg.ROT, i, cs]
            sw = ropep.tile([32, SW], BF16, tag="swap", name="sw")
            nc.vector.stream_shuffle(sw, blk, rope_mask)
            with nc.allow_low_precision(reason="bf16 rope"):
                nc.vector.tensor_mul(sw, sw, sin_sb[:, cs])
                nc.vector.tensor_mul(blk, blk, cos_sb[:, cs])
                nc.vector.tensor_add(blk, blk, sw)

        # ---------------- Phase 1: QKV projection ------------------------
        with tc.tile_pool(name="wq", bufs=1) as wp, \
             tc.tile_pool(name="xs", bufs=2) as xp, \
             tc.tile_pool(name="ps1", bufs=2, space="PSUM") as pp1:
            w_sb = wp.tile([128, KT, cfg.WCOLS], BF16)
            w_view = wcat.rearrange("(kt p) r -> p kt r", p=128)
            x_view = xT.rearrange("(kt p) s -> p kt s", p=128)

            # interleave per-kt w loads with quarters of the first x chunk
            xt0 = xp.tile([128, KT, SC], BF16, tag="xt")
            ktg = max(1, KT // 4)
            for q0 in range(0, KT, ktg):
                q1 = min(q0 + ktg, KT)
                for kt in range(q0, q1):
                    nc.sync.dma_start(out=w_sb[:, kt, :], in_=w_view[:, kt, :])
                nc.sync.dma_start(out=xt0[:, q0:q1, :],
                                  in_=x_view[:, q0:q1, 0:SC])
            xt1 = xp.tile([128, KT, SC], BF16, tag="xt")
            nc.sync.dma_start(out=xt1[:, :, :], in_=x_view[:, :, SC:2 * SC])

            # constants (after the critical w/x stream)
            ident = consts.tile([128, 128], F32)
            make_identity(nc, ident)
            identB = consts.tile([128, 128], BF16)
            with nc.allow_low_precision(reason="bf16 identity"):
                nc.vector.tensor_copy(identB, ident)
            ones_tmp = consts.tile([128, 1], BF16, tag="onestmp")
            nc.vector.memset(ones_tmp, 1.0)
            ones_col = ones_tmp
            mask_sb = consts.tile([128, 128], BF16)
            nc.sync.dma_start(out=mask_sb, in_=maskT[:, :])
            bqk_sb = consts.tile([128, NQK], F32)
            nc.sync.dma_start(out=bqk_sb,
                              in_=bqk.rearrange("(rt p) -> p rt", p=128))
            vb_sb = consts.tile([128, VW], F32)
            nc.sync.dma_start(out=vb_sb, in_=vbbc[:, :])
            cos_sb = consts.tile([32, SF], BF16, tag="costab")
            sin_sb = consts.tile([32, SF], BF16, tag="sintab")
            nc.sync.dma_start(out=cos_sb, in_=cosT[:, :])
            nc.sync.dma_start(out=sin_sb, in_=sinT[:, :])
            wd_sb = consts.tile([128, CT, E], BF16, tag="wd")
            nc.sync.dma_start(
                out=wd_sb[:, :, :],
                in_=wdT.rearrange("(ct p) e -> p ct e", p=128))

            ntile = SC // 128   # v token sub-tiles per chunk (=2)
            for sc in range(cfg.NP1):
                if sc == 0:
                    xt = xt0
                elif sc == 1:
                    xt = xt1
                else:
                    xt = xp.tile([128, KT, SC], BF16, tag="xt")
                    nc.sync.dma_start(
                        out=xt[:, :, :],
                        in_=x_view[:, :, sc * SC:(sc + 1) * SC])
                qkps = [pp1.tile([128, 512], F32, tag=f"qk{i}", name=f"qkps{i}")
                        for i in range(NQK // 2)]
                vps = pp1.tile([128, 512], F32, tag="v")
                # NOTE: a start=True matmul marks the PSUM bank's whole 2KB
                # zero-region pending-zero, so only the FIRST matmul into
                # each bank starts; co-resident column groups accumulate
                # onto pending-zero bytes.
                for kt in range(KT):
                    fl, ll = (kt == 0), (kt == KT - 1)
                    for i in range(NQK):
                        nc.tensor.matmul(
                            qkps[i // 2][:, 256 * (i % 2):256 * (i % 2) + SC],
                            w_sb[:, kt, 128 * i:128 * (i + 1)],
                            xt[:, kt, :],
                            start=fl and i % 2 == 0,
                            stop=ll and i % 2 == 1, skip_group_check=True)
                    for t in range(ntile):
                        nc.tensor.matmul(
                            vps[:, VW * t:VW * (t + 1)],
                            xt[:, kt, 128 * t:128 * (t + 1)],
                            w_sb[:, kt, RW:RW + VW],
                            start=fl and t == 0,
                            stop=ll and t == ntile - 1, skip_group_check=True)
                for i in range(NQK):
                    nc.scalar.activation(
                        qk_sb[:, i, sc * SC:(sc + 1) * SC],
                        qkps[i // 2][:, 256 * (i % 2):256 * (i % 2) + SC],
                        AF.Identity, bias=bqk_sb[:, i:i + 1])
                with nc.allow_low_precision(reason="bf16 v eviction"):
                    for t in range(ntile):
                        nc.vector.tensor_add(
                            vnat[:, sc * ntile + t, :],
                            vps[:, VW * t:VW * (t + 1)], vb_sb)
                if (sc + 1) % (SW // SC) == 0:
                    for i in range(NQK):
                        rope_slice(i, sc // (SW // SC))

        # ---------------- Attention + interleaved dense -------------------
        feeder = _Feeder()
        with tc.tile_pool(name="pstrip", bufs=3) as ptp, \
             tc.tile_pool(name="norm", bufs=2) as npool, \
             tc.tile_pool(name="outsb", bufs=4) as op, \
             tc.tile_pool(name="psA", bufs=3, space="PSUM") as psA, \
             tc.tile_pool(name="psY", bufs=2, space="PSUM") as psY, \
             tc.tile_pool(name="psS", bufs=1, space="PSUM") as psS, \
             tc.tile_pool(name="psD", bufs=2, space="PSUM") as psD:

            def dense_steps(b, scp):
                # one (eo) output row-tile over two 512-token col chunks
                for eo in range(EO):
                    ot = op.tile([128, 1024], BF16, tag="out")
                    for t in range(2):
                        col = b * S + (2 * scp + t) * 512
                        pd = psD.tile([128, 512], F32, tag="D")
                        for ct in range(CT):
                            nc.tensor.matmul(
                                pd,
                                wd_sb[:, ct, 128 * eo:128 * (eo + 1)],
                                yT_sb[:, ct, col:col + 512],
                                start=(ct == 0), stop=(ct == CT - 1),
                                skip_group_check=True)
                        yield
                        # evictions stay off ScalarE: exp saturates it
                        with nc.allow_low_precision(reason="bf16 out"):
                            nc.vector.tensor_copy(
                                ot[:, 512 * t:512 * (t + 1)], pd)
                    nc.sync.dma_start(
                        out=outT[128 * eo:128 * (eo + 1),
                                 b * S + scp * 1024:b * S + (scp + 1) * 1024],
                        in_=ot)
                    yield

            def attention(b, hl, on_chain=None):
                scol = b * S
                q_t = qk_sb[:, 2 * hl, scol:scol + S]
                k_t = qk_sb[:, 2 * hl + 1, scol:scol + S]
                pending = [None]   # chunk-end normalization closure

                def emit_chain(c, psYt, psSt):
                    recip = npool.tile([128, 4], BF16, tag="recip")
                    with nc.allow_low_precision(reason="bf16 recip"):
                        nc.vector.reciprocal(recip, psSt[:, 0:4])
                    # transpose each recip column to partition 0 ([1, 128])
                    rps = psA.tile([128, 512], BF16, tag="A", name="rps")
                    for g in range(4):
                        nc.tensor.matmul(
                            rps[0:1, 128 * g:128 * (g + 1)],
                            recip[:, g:g + 1], identB,
                            is_transpose=True, start=(g == 0), stop=(g == 3),
                            skip_group_check=True)
                    rT = npool.tile([1, 512], F32, tag="rT")
                    nc.vector.tensor_copy(rT, rps[0:1, 0:512])
                    bc = npool.tile([128, 512], F32, tag="bc")
                    for g in range(4):
                        nc.gpsimd.partition_broadcast(
                            bc[:, 128 * g:128 * (g + 1)],
                            rT[0:1, 128 * g:128 * (g + 1)])
                    with nc.allow_low_precision(reason="bf16 y eviction"):
                        nc.vector.tensor_mul(
                            yT_sb[:, hl, scol + c * 512:scol + (c + 1) * 512],
                            psYt[:, 0:512], bc)
                    if on_chain is not None:
                        on_chain(c)

                for c in range(cfg.NQC):
                    nj = 4 * (c + 1)
                    psYt = psY.tile([128, 512], F32, tag="Y")
                    psSt = psS.tile([128, 4], F32, tag="S")
                    prev = None

                    def emit_ys(j, pT, off, g0, psYt=psYt, psSt=psSt, c=c,
                                nj=nj):
                        nc.tensor.matmul(
                            psYt[:, off:512],
                            vnat[:, b * (S // 128) + j, 128 * hl:128 * (hl + 1)],
                            pT[:, off:512],
                            start=(j == 0), stop=(j == nj - 1),
                            skip_group_check=True)
                        for g in range(g0, 4):
                            # start only on the very first sums matmul of the
                            # chunk (bank-wide zero region); later columns
                            # accumulate onto pending-zero bytes.
                            nc.tensor.matmul(
                                psSt[:, g:g + 1],
                                pT[:, 128 * g:128 * (g + 1)], ones_col,
                                start=(j == 0 and g == 0),
                                stop=(j == nj - 1 and g == 3),
                                skip_group_check=True)

                    for j in range(nj):
                        g0 = max(0, j - 4 * c)
                        off = 128 * g0
                        ps = psA.tile([128, 512], F32, tag="A")
                        nc.tensor.matmul(
                            ps[:, off:512],
                            k_t[:, 128 * j:128 * (j + 1)],
                            q_t[:, c * 512 + off:(c + 1) * 512],
                            start=True, stop=True, skip_group_check=True)
                        pT = ptp.tile([128, 512], BF16, tag="p")
                        nc.scalar.activation(
                            pT[:, off:512], ps[:, off:512], AF.Exp,
                            scale=cfg.SCALE)
                        if j >= 4 * c:
                            # causal mask as a cheap post-exp 0/1 multiply
                            with nc.allow_low_precision(reason="bf16 mask"):
                                nc.vector.tensor_mul(
                                    pT[:, off:off + 128],
                                    pT[:, off:off + 128], mask_sb)
                        if prev is not None:
                            emit_ys(*prev)
                        if j == 0 and pending[0] is not None:
                            pending[0]()
                            pending[0] = None
                        feeder.step()
                        prev = (j, pT, off, g0)
                    emit_ys(*prev)
                    feeder.step()
                    pending[0] = (lambda c=c, y=psYt, s=psSt:
                                  emit_chain(c, y, s))
                if pending[0] is not None:
                    pending[0]()
                    pending[0] = None

            for b in range(B):
                for hl in range(HPC):
                    if hl == HPC - 1:
                        # dense cols [0:1024*(scp+1)) ready once this head's
                        # chunk 2*scp+1 is normalized
                        hook = (lambda c, b=b: feeder.push(
                            dense_steps(b, (c - 1) // 2)) if c % 2 == 1
                            else None)
                    else:
                        hook = None
                    attention(b, hl, on_chain=hook)
            feeder.drain()

            if debug:
                dqk = nc.dram_tensor("dbg_qk", [128, NQK, SF], BF16,
                                     kind="ExternalOutput")
                dv = nc.dram_tensor("dbg_v", [128, NT, VW], BF16,
                                    kind="ExternalOutput")
                dy = nc.dram_tensor("dbg_y", [128, CT, SF], BF16,
                                    kind="ExternalOutput")
                nc.sync.dma_start(out=dqk[:, :, :], in_=qk_sb[:, :, :])
                nc.sync.dma_start(out=dv[:, :, :], in_=vnat[:, :, :])
                nc.sync.dma_start(out=dy[:, :, :], in_=yT_sb[:, :, :])

    nc.finalize()
    return nc


# ---------------------------------------------------------------------------
# Host-side input preparation / sharding
# ---------------------------------------------------------------------------

def _bf16(a):
    import ml_dtypes
    return np.ascontiguousarray(a, np.float32).astype(ml_dtypes.bfloat16)


def _rope_tables(cfg: Cfg):
    inv_freq = 1.0 / (10000.0 ** (np.arange(0, cfg.ROT, 2, dtype=np.float64)
                                  / cfg.ROT))
    t = np.arange(cfg.S, dtype=np.float64)
    freqs = np.outer(t, inv_freq)                       # [S, 16]
    emb = np.concatenate([freqs, freqs], axis=-1)       # [S, 32]
    cos = np.cos(emb).T.astype(np.float32)              # [32, S]
    sin = np.sin(emb).T.astype(np.float32)
    cosF = np.tile(cos, (1, cfg.B))                     # [32, SF]
    sinF = np.tile(sin, (1, cfg.B))
    sinF[:cfg.ROT // 2] *= -1.0                         # fold rotate_half sign
    return _bf16(cosF), _bf16(sinF)


def make_in_maps(cfg: Cfg, x, w_qkv, b_qkv, w_dense):
    HS, HPC = cfg.HS, cfg.HPC
    xTb = _bf16(np.ascontiguousarray(
        np.asarray(x, np.float32).reshape(cfg.SF, cfg.E).T))
    cos128, sin128s = _rope_tables(cfg)
    p = np.arange(128)[:, None]
    f = np.arange(128)[None, :]
    maskT = _bf16(np.where(p <= f, 1.0, 0.0))   # post-exp 0/1 causal mask
    in_maps = []
    for i in range(cfg.n_cores):
        heads = [HPC * i + h for h in range(HPC)]
        qk_rows = np.concatenate(
            [np.arange(h * 3 * HS + qk * HS, h * 3 * HS + (qk + 1) * HS)
             for h in heads for qk in range(2)])
        v_rows = np.concatenate(
            [np.arange(h * 3 * HS + 2 * HS, h * 3 * HS + 3 * HS)
             for h in heads])
        wcat = np.concatenate(
            [np.asarray(w_qkv, np.float32)[qk_rows, :].T,
             np.asarray(w_qkv, np.float32)[v_rows, :].T], axis=1)
        cols = slice(i * cfg.VW, (i + 1) * cfg.VW)
        in_maps.append({
            "xT": xTb,
            "wcat": _bf16(wcat),
            "bqk": np.ascontiguousarray(
                np.asarray(b_qkv, np.float32)[qk_rows]),
            "vbbc": np.ascontiguousarray(np.tile(
                np.asarray(b_qkv, np.float32)[v_rows][None, :], (128, 1))),
            "wdT": _bf16(np.asarray(w_dense, np.float32)[:, cols].T),
            "cosT": cos128,
            "sinT": sin128s,
            "maskT": maskT,
        })
    return in_maps


def combine_outputs(cfg: Cfg, results, b_dense):
    acc = np.zeros((cfg.E, cfg.SF), dtype=np.float64)
    for r in results:
        acc += np.asarray(r["outT"]).astype(np.float64)
    out = acc.T.reshape(cfg.B, cfg.S, cfg.E) + \
        np.asarray(b_dense, np.float64)
    return out.astype(np.float32)


_PROGRAM_CACHE = {}


def kernel(x, w_qkv, b_qkv, w_dense, b_dense):
    from concourse.bass_utils import run_bass_kernel_spmd

    cfg = Cfg()
    key = "full"
    if key not in _PROGRAM_CACHE:
        _PROGRAM_CACHE[key] = build_program(cfg)
    nc = _PROGRAM_CACHE[key]
    in_maps = make_in_maps(cfg, np.asarray(x), np.asarray(w_qkv),
                           np.asarray(b_qkv), np.asarray(w_dense))
    res = run_bass_kernel_spmd(nc, in_maps, list(range(cfg.n_cores)))
    return combine_outputs(cfg, res.results, np.asarray(b_dense))


# revision 45
# speedup vs baseline: 1.0732x; 1.0442x over previous
"""GPT-NeoX attention layer (B=2, S=2048, E=2048, H=16, partial RoPE 32/128)
as a Bass/Tile kernel for 8 Trainium2 NeuronCores.

Sharding: tensor-parallel across heads (2 heads per core, Megatron-style).
Each core computes QKV projection for its 2 heads over all tokens, applies
partial RoPE, runs causal attention, and produces a partial dense output
(contraction over its 256 columns of w_dense).  The 8 bf16 partial outputs
are summed on the host and the dense bias is added once on the host.

Everything on-device is bf16 (inputs pre-converted on the host); PSUM
accumulation stays fp32.  Key structure choices:

  qk_sb  [128, 4, SF]   Q^T/K^T per head (head dim on partitions) - scores
                        and y^T matmuls consume this directly.
  vnat   [128, SF/128, 256]  V in NATURAL [token, d] layout, produced in
                        phase 1 by x-stationary matmuls (x as lhsT), so no
                        PE transposes of V are ever needed.
  scores S^T = K^T.T @ Q^T in [sk, sq] blocks; exp on ScalarE (pipelined one
                        block behind the scores matmuls).
  softmax sums          via N=1 matmuls with the exp'd block as the
                        stationary operand (out [sq,1]): nearly free on PE,
                        instead of a 512-wide ones-matmul per block.
  normalize             reciprocal -> tiny PE transpose -> GPSIMD
                        partition_broadcast -> one DVE multiply per chunk.
  dense                 interleaved into later attention heads (fills the
                        tensor engine while ScalarE works through exp).
"""

import numpy as np
from contextlib import ExitStack

import concourse.bass as bass
import concourse.bacc as bacc
import concourse.mybir as mybir
import concourse.tile as tile
from concourse.masks import make_identity

AF = mybir.ActivationFunctionType
F32 = mybir.dt.float32
BF16 = mybir.dt.bfloat16

NEG_MASK = -1.0e9


class Cfg:
    def __init__(self, B=2, S=2048, E=2048, H=16, n_cores=8):
        self.B, self.S, self.E, self.H = B, S, E, H
        self.HS = 128                  # head size (one partition tile)
        self.ROT = 32                  # rotary dims
        self.n_cores = n_cores
        self.HPC = H // n_cores        # heads per core
        assert self.HPC == 2, "kernel assumes 2 heads per core"
        self.NQK = 2 * self.HPC        # q/k row tiles (h0q,h0k,h1q,h1k)
        self.VW = self.HPC * self.HS   # v natural width (d per core)
        self.RW = self.NQK * self.HS   # per-core q+k rows
        self.WCOLS = self.RW + self.VW
        self.SF = B * S
        self.KT = E // 128             # contraction tiles
        self.SC = 256                  # phase-1 token chunk
        self.NP1 = self.SF // self.SC
        self.G = self.SF // 4          # rope regroup width
        self.NQC = S // 512            # q chunks per (b, h)
        self.EO = E // 128             # dense output row tiles
        self.CT = self.HPC             # dense contraction tiles
        self.SCALE = 1.0 / np.sqrt(self.HS)
        assert S % 512 == 0 and E % 128 == 0 and self.SF % (4 * self.SC) == 0


class _Feeder:
    """FIFO of deferred dense micro-step generators, materialized from
    (b, scp) specs once `factory` is set and the feeder is enabled."""

    def __init__(self):
        self.specs = []
        self.gens = []
        self.factory = None
        self.enabled = False

    def push(self, spec):
        self.specs.append(spec)

    def _refill(self):
        if not self.gens and self.specs and self.factory:
            self.gens.append(self.factory(*self.specs.pop(0)))

    def step(self):
        if not self.enabled:
            return
        self._refill()
        while self.gens:
            try:
                next(self.gens[0])
                return
            except StopIteration:
                self.gens.pop(0)
                self._refill()

    def drain(self):
        assert self.enabled
        while True:
            self._refill()
            if not self.gens:
                return
            for _ in self.gens.pop(0):
                pass


class _Pump:
    """Steps an attention generator one j-block at a time; the generator
    yields an int (p1 chunks that must be emitted first) before each chunk
    and None per block."""

    def __init__(self, gen):
        self.gen = gen
        self.parked = None
        self.done = False

    def step(self, sc):
        if self.done:
            return False
        if self.parked is not None:
            if self.parked > sc:
                return False
            self.parked = None
        while True:
            try:
                v = next(self.gen)
            except StopIteration:
                self.done = True
                return False
            if v is None:
                return True
            if v > sc:
                self.parked = v
                return False


def build_program(cfg: Cfg, debug: bool = False) -> bass.Bass:
    B, S, E = cfg.B, cfg.S, cfg.E
    SF, KT, G = cfg.SF, cfg.KT, cfg.G
    SC, NQK, VW, RW = cfg.SC, cfg.NQK, cfg.VW, cfg.RW
    HPC, CT, EO = cfg.HPC, cfg.CT, cfg.EO
    NT = SF // 128                   # vnat token tiles

    nc = bacc.Bacc(None)
    xT = nc.dram_tensor("xT", [E, SF], BF16, kind="ExternalInput")
    wcat = nc.dram_tensor("wcat", [E, cfg.WCOLS], BF16, kind="ExternalInput")
    bqk = nc.dram_tensor("bqk", [RW], F32, kind="ExternalInput")
    vbbc = nc.dram_tensor("vbbc", [128, VW], F32, kind="ExternalInput")
    wdT = nc.dram_tensor("wdT", [VW, E], BF16, kind="ExternalInput")
    cosT = nc.dram_tensor("cosT", [32, SF], BF16, kind="ExternalInput")
    sinT = nc.dram_tensor("sinT", [32, SF], BF16, kind="ExternalInput")
    maskT = nc.dram_tensor("maskT", [128, 128], BF16, kind="ExternalInput")
    outT = nc.dram_tensor("outT", [E, SF], BF16, kind="ExternalOutput")

    with tile.TileContext(nc) as tc, ExitStack() as stk:
        consts = stk.enter_context(tc.tile_pool(name="consts", bufs=1))
        bigp = stk.enter_context(tc.tile_pool(name="big", bufs=1))
        qk_sb = bigp.tile([128, NQK, SF], BF16)
        vnat = bigp.tile([128, NT, VW], BF16)
        yT_sb = bigp.tile([128, CT, SF], BF16)

        # constants (tiles declared here; filled during phase-1 emission,
        # after the critical w/x DMA stream is issued)
        ident = consts.tile([128, 128], F32)
        identB = consts.tile([128, 128], BF16)
        ones_col = consts.tile([128, 1], BF16)
        mask_sb = consts.tile([128, 128], BF16)
        bqk_sb = consts.tile([128, NQK], F32)
        vb_sb = consts.tile([128, VW], F32)
        cos_sb = consts.tile([32, SF], BF16, tag="costab")
        sin_sb = consts.tile([32, SF], BF16, tag="sintab")
        wd_sb = consts.tile([128, CT, E], BF16, tag="wd")

        # RoPE: rotate_half is a partition swap within the 32 rot rows ->
        # DVE stream_shuffle (per-quadrant permutation) + elementwise
        # combine in [32, cols] layout, zero DMAs.  Each 1024-col slice is
        # emitted as soon as the phase-1 chunks covering it are done.
        SW = 1024
        rope_mask = [(i + 16) % 32 for i in range(32)]
        ropep = stk.enter_context(tc.tile_pool(name="rope", bufs=2))

        def rope_slice(i, sl):
            cs = slice(sl * SW, (sl + 1) * SW)
            blk = qk_sb[0:cfg.ROT, i, cs]
            sw = ropep.tile([32, SW], BF16, tag="swap", name="sw")
            nc.vector.stream_shuffle(sw, blk, rope_mask)
            with nc.allow_low_precision(reason="bf16 rope"):
                nc.vector.tensor_mul(sw, sw, sin_sb[:, cs])
                nc.vector.tensor_mul(blk, blk, cos_sb[:, cs])
                nc.vector.tensor_add(blk, blk, sw)

        # ---------------- Attention pools (live through phase 1) ----------
        feeder = _Feeder()
        ptp = stk.enter_context(tc.tile_pool(name="pstrip", bufs=6))
        npool = stk.enter_context(tc.tile_pool(name="norm", bufs=2))
        psA = stk.enter_context(tc.tile_pool(name="psA", bufs=2, space="PSUM"))
        psY = stk.enter_context(tc.tile_pool(name="psY", bufs=2, space="PSUM"))
        psS = stk.enter_context(tc.tile_pool(name="psS", bufs=1, space="PSUM"))
        LAG = 3   # j-blocks between scores+exp emission and yacc+sums

        def attention(b, hl, on_chain=None):
            """Generator: yields the p1-chunk prerequisite (int) before each
            q-chunk, then None after each emitted j-block."""
            scol = b * S
            q_t = qk_sb[:, 2 * hl, scol:scol + S]
            k_t = qk_sb[:, 2 * hl + 1, scol:scol + S]

            def emit_chain(c, psYt, psSt):
                recip = npool.tile([128, 4], BF16, tag="recip")
                with nc.allow_low_precision(reason="bf16 recip"):
                    nc.vector.reciprocal(recip, psSt[:, 0:4])
                # transpose each recip column to partition 0 ([1, 128])
                rps = psA.tile([128, 512], BF16, tag="A", name="rps")
                for g in range(4):
                    nc.tensor.matmul(
                        rps[0:1, 128 * g:128 * (g + 1)],
                        recip[:, g:g + 1], identB,
                        is_transpose=True, start=(g == 0), stop=(g == 3),
                        skip_group_check=True)
                rT = npool.tile([1, 512], F32, tag="rT")
                nc.vector.tensor_copy(rT, rps[0:1, 0:512])
                bc = npool.tile([128, 512], F32, tag="bc")
                for g in range(4):
                    nc.gpsimd.partition_broadcast(
                        bc[:, 128 * g:128 * (g + 1)],
                        rT[0:1, 128 * g:128 * (g + 1)])
                with nc.allow_low_precision(reason="bf16 y eviction"):
                    nc.vector.tensor_mul(
                        yT_sb[:, hl, scol + c * 512:scol + (c + 1) * 512],
                        psYt[:, 0:512], bc)
                if on_chain is not None:
                    on_chain(c)

            for c in range(cfg.NQC):
                # p1 chunks needed: data cols + one extra chunk so the rope
                # DVE work emitted at the slice boundary is already done
                sl_needed = (b * S + 512 * (c + 1) - 1) // SW
                yield (sl_needed + 1) * (SW // SC) + hl
                nj = 4 * (c + 1)
                psYt = psY.tile([128, 512], F32, tag="Y")
                psSt = psS.tile([128, 4], F32, tag="S")
                pend = []

                def emit_ys(j, pT, off, g0, psYt=psYt, psSt=psSt, c=c,
                            nj=nj):
                    nc.tensor.matmul(
                        psYt[:, off:512],
                        vnat[:, b * (S // 128) + j, 128 * hl:128 * (hl + 1)],
                        pT[:, off:512],
                        start=(j == 0), stop=(j == nj - 1),
                        skip_group_check=True)
                    for g in range(g0, 4):
                        # start only on the very first sums matmul of the
                        # chunk (bank-wide zero region); later columns
                        # accumulate onto pending-zero bytes.
                        nc.tensor.matmul(
                            psSt[:, g:g + 1],
                            pT[:, 128 * g:128 * (g + 1)], ones_col,
                            start=(j == 0 and g == 0),
                            stop=(j == nj - 1 and g == 3),
                            skip_group_check=True)

                for j in range(nj):
                    g0 = max(0, j - 4 * c)
                    off = 128 * g0
                    ps = psA.tile([128, 512], F32, tag="A")
                    nc.tensor.matmul(
                        ps[:, off:512],
                        k_t[:, 128 * j:128 * (j + 1)],
                        q_t[:, c * 512 + off:(c + 1) * 512],
                        start=True, stop=True, skip_group_check=True)
                    pT = ptp.tile([128, 512], BF16, tag=f"p{hl}", name="pT")
                    nc.scalar.activation(
                        pT[:, off:512], ps[:, off:512], AF.Exp,
                        scale=cfg.SCALE)
                    if j >= 4 * c:
                        # causal mask as a cheap post-exp 0/1 multiply
                        with nc.allow_low_precision(reason="bf16 mask"):
                            nc.vector.tensor_mul(
                                pT[:, off:off + 128],
                                pT[:, off:off + 128], mask_sb)
                    if len(pend) >= LAG:
                        emit_ys(*pend.pop(0))
                    feeder.step()
                    pend.append((j, pT, off, g0))
                    yield None
                while pend:
                    emit_ys(*pend.pop(0))
                feeder.step()
                # chunk-end chain emitted atomically (the single psS bank
                # must be read here before another head's sums start)
                emit_chain(c, psYt, psSt)

        def make_pump(b, hl):
            hook = None
            if hl == HPC - 1:
                def hook(c, b=b):
                    if c % 2 == 1:
                        feeder.push((b, (c - 1) // 2))
            return _Pump(attention(b, hl, on_chain=hook))

        pumps_b0 = [make_pump(0, hl) for hl in range(HPC)]
        slot_budget = [0]
        active = [0]

        def inject(sc):
            # Called at points inside phase-1 emission; injects one ready
            # attention j-block into the PE stream.  Sticky generator choice:
            # a generator only loses its turn at a chunk boundary, so pool
            # tiles shared across generators (psS, psY) stay chunk-atomic.
            if slot_budget[0] <= 0:
                return
            n = len(pumps_b0)
            for off in range(n):
                p = pumps_b0[(active[0] + off) % n]
                if p.step(sc):
                    active[0] = (active[0] + off) % n
                    slot_budget[0] -= 1
                    return

        # ---------------- Phase 1: QKV projection ------------------------
        with tc.tile_pool(name="wq", bufs=1) as wp, \
             tc.tile_pool(name="xs", bufs=2) as xp, \
             tc.tile_pool(name="ps1", bufs=1, space="PSUM") as pp1:
            w_sb = wp.tile([128, KT, cfg.WCOLS], BF16)
            w_view = wcat.rearrange("(kt p) r -> p kt r", p=128)
            x_view = xT.rearrange("(kt p) s -> p kt s", p=128)

            # interleave per-kt w loads with quarters of the first x chunk
            xt0 = xp.tile([128, KT, SC], BF16, tag="xt")
            ktg = max(1, KT // 4)
            for q0 in range(0, KT, ktg):
                q1 = min(q0 + ktg, KT)
                for kt in range(q0, q1):
                    nc.sync.dma_start(out=w_sb[:, kt, :], in_=w_view[:, kt, :])
                nc.sync.dma_start(out=xt0[:, q0:q1, :],
                                  in_=x_view[:, q0:q1, 0:SC])
            xt1 = xp.tile([128, KT, SC], BF16, tag="xt")
            nc.sync.dma_start(out=xt1[:, :, :], in_=x_view[:, :, SC:2 * SC])

            # constants (after the critical w/x stream)
            make_identity(nc, ident)
            with nc.allow_low_precision(reason="bf16 identity"):
                nc.vector.tensor_copy(identB, ident)
            nc.vector.memset(ones_col, 1.0)
            nc.sync.dma_start(out=mask_sb, in_=maskT[:, :])
            nc.sync.dma_start(out=bqk_sb,
                              in_=bqk.rearrange("(rt p) -> p rt", p=128))
            nc.sync.dma_start(out=vb_sb, in_=vbbc[:, :])
            nc.sync.dma_start(out=cos_sb, in_=cosT[:, :])
            nc.sync.dma_start(out=sin_sb, in_=sinT[:, :])
            nc.sync.dma_start(
                out=wd_sb[:, :, :],
                in_=wdT.rearrange("(ct p) e -> p ct e", p=128))

            ntile = SC // 128   # v token sub-tiles per chunk (=2)
            # group-major chunks: 6 sequential accumulation groups cycling
            # 3 single-buffered banks (paired groups share a bank via the
            # pending-zero trick), evicted inline as each group finishes.
            for sc in range(cfg.NP1):
                slot_budget[0] = 7
                if sc == 0:
                    xt = xt0
                elif sc == 1:
                    xt = xt1
                else:
                    xt = xp.tile([128, KT, SC], BF16, tag="xt")
                    nc.sync.dma_start(
                        out=xt[:, :, :],
                        in_=x_view[:, :, sc * SC:(sc + 1) * SC])
                for pair in range(3):
                    pt = pp1.tile([128, 512], F32, tag=f"p1{pair}",
                                  name=f"p1ps{pair}")
                    for half in range(2):
                        grp = 2 * pair + half
                        reg = pt[:, 256 * half:256 * half + 256]
                        for kt in range(KT):
                            fl, ll = (kt == 0), (kt == KT - 1)
                            if grp < NQK:
                                nc.tensor.matmul(
                                    reg,
                                    w_sb[:, kt, 128 * grp:128 * (grp + 1)],
                                    xt[:, kt, :],
                                    start=fl and half == 0,
                                    stop=ll and half == 1,
                                    skip_group_check=True)
                            else:
                                t = grp - NQK
                                nc.tensor.matmul(
                                    reg,
                                    xt[:, kt, 128 * t:128 * (t + 1)],
                                    w_sb[:, kt, RW:RW + VW],
                                    start=fl and half == 0,
                                    stop=ll and half == 1,
                                    skip_group_check=True)
                            if kt % 4 == 3:
                                inject(sc - 1)
                        # inline eviction of this group
                        if grp < NQK:
                            nc.scalar.activation(
                                qk_sb[:, grp, sc * SC:(sc + 1) * SC], reg,
                                AF.Identity, bias=bqk_sb[:, grp:grp + 1])
                        else:
                            t = grp - NQK
                            with nc.allow_low_precision(reason="bf16 v"):
                                nc.vector.tensor_add(
                                    vnat[:, sc * ntile + t, :], reg, vb_sb)
                if (sc + 1) % (SW // SC) == 0:
                    for i in range(NQK):
                        rope_slice(i, sc // (SW // SC))

        # ---------------- Tail: remaining attention + dense ----------------
        with tc.tile_pool(name="outsb", bufs=4) as op, \
             tc.tile_pool(name="psD", bufs=2, space="PSUM") as psD:

            def dense_steps(b, scp):
                # one (eo) output row-tile over two 512-token col chunks
                for eo in range(EO):
                    ot = op.tile([128, 1024], BF16, tag="out")
                    for t in range(2):
                        col = b * S + (2 * scp + t) * 512
                        pd = psD.tile([128, 512], F32, tag="D")
                        for ct in range(CT):
                            nc.tensor.matmul(
                                pd,
                                wd_sb[:, ct, 128 * eo:128 * (eo + 1)],
                                yT_sb[:, ct, col:col + 512],
                                start=(ct == 0), stop=(ct == CT - 1),
                                skip_group_check=True)
                        yield
                        # evictions stay off ScalarE: exp saturates it
                        with nc.allow_low_precision(reason="bf16 out"):
                            nc.vector.tensor_copy(
                                ot[:, 512 * t:512 * (t + 1)], pd)
                    nc.sync.dma_start(
                        out=outT[128 * eo:128 * (eo + 1),
                                 b * S + scp * 1024:b * S + (scp + 1) * 1024],
                        in_=ot)
                    yield

            feeder.factory = dense_steps
            feeder.enabled = True
            BIG = 10 ** 9
            while any(p.step(BIG) for p in pumps_b0):
                pass
            for bb in range(1, B):
                pumps = [make_pump(bb, hl) for hl in range(HPC)]
                for p in pumps:
                    while p.step(BIG):
                        pass
            feeder.drain()

        if debug:
            dqk = nc.dram_tensor("dbg_qk", [128, NQK, SF], BF16,
                                 kind="ExternalOutput")
            dv = nc.dram_tensor("dbg_v", [128, NT, VW], BF16,
                                kind="ExternalOutput")
            dy = nc.dram_tensor("dbg_y", [128, CT, SF], BF16,
                                kind="ExternalOutput")
            nc.sync.dma_start(out=dqk[:, :, :], in_=qk_sb[:, :, :])
            nc.sync.dma_start(out=dv[:, :, :], in_=vnat[:, :, :])
            nc.sync.dma_start(out=dy[:, :, :], in_=yT_sb[:, :, :])

    nc.finalize()
    return nc


# ---------------------------------------------------------------------------
# Host-side input preparation / sharding
# ---------------------------------------------------------------------------

def _bf16(a):
    import ml_dtypes
    return np.ascontiguousarray(a, np.float32).astype(ml_dtypes.bfloat16)


def _rope_tables(cfg: Cfg):
    inv_freq = 1.0 / (10000.0 ** (np.arange(0, cfg.ROT, 2, dtype=np.float64)
                                  / cfg.ROT))
    t = np.arange(cfg.S, dtype=np.float64)
    freqs = np.outer(t, inv_freq)                       # [S, 16]
    emb = np.concatenate([freqs, freqs], axis=-1)       # [S, 32]
    cos = np.cos(emb).T.astype(np.float32)              # [32, S]
    sin = np.sin(emb).T.astype(np.float32)
    cosF = np.tile(cos, (1, cfg.B))                     # [32, SF]
    sinF = np.tile(sin, (1, cfg.B))
    sinF[:cfg.ROT // 2] *= -1.0                         # fold rotate_half sign
    return _bf16(cosF), _bf16(sinF)


def make_in_maps(cfg: Cfg, x, w_qkv, b_qkv, w_dense):
    HS, HPC = cfg.HS, cfg.HPC
    xTb = _bf16(np.ascontiguousarray(
        np.asarray(x, np.float32).reshape(cfg.SF, cfg.E).T))
    cos128, sin128s = _rope_tables(cfg)
    p = np.arange(128)[:, None]
    f = np.arange(128)[None, :]
    maskT = _bf16(np.where(p <= f, 1.0, 0.0))   # post-exp 0/1 causal mask
    in_maps = []
    for i in range(cfg.n_cores):
        heads = [HPC * i + h for h in range(HPC)]
        qk_rows = np.concatenate(
            [np.arange(h * 3 * HS + qk * HS, h * 3 * HS + (qk + 1) * HS)
             for h in heads for qk in range(2)])
        v_rows = np.concatenate(
            [np.arange(h * 3 * HS + 2 * HS, h * 3 * HS + 3 * HS)
             for h in heads])
        wcat = np.concatenate(
            [np.asarray(w_qkv, np.float32)[qk_rows, :].T,
             np.asarray(w_qkv, np.float32)[v_rows, :].T], axis=1)
        cols = slice(i * cfg.VW, (i + 1) * cfg.VW)
        in_maps.append({
            "xT": xTb,
            "wcat": _bf16(wcat),
            "bqk": np.ascontiguousarray(
                np.asarray(b_qkv, np.float32)[qk_rows]),
            "vbbc": np.ascontiguousarray(np.tile(
                np.asarray(b_qkv, np.float32)[v_rows][None, :], (128, 1))),
            "wdT": _bf16(np.asarray(w_dense, np.float32)[:, cols].T),
            "cosT": cos128,
            "sinT": sin128s,
            "maskT": maskT,
        })
    return in_maps


def combine_outputs(cfg: Cfg, results, b_dense):
    acc = np.zeros((cfg.E, cfg.SF), dtype=np.float64)
    for r in results:
        acc += np.asarray(r["outT"]).astype(np.float64)
    out = acc.T.reshape(cfg.B, cfg.S, cfg.E) + \
        np.asarray(b_dense, np.float64)
    return out.astype(np.float32)


_PROGRAM_CACHE = {}


def kernel(x, w_qkv, b_qkv, w_dense, b_dense):
    from concourse.bass_utils import run_bass_kernel_spmd

    cfg = Cfg()
    key = "full"
    if key not in _PROGRAM_CACHE:
        _PROGRAM_CACHE[key] = build_program(cfg)
    nc = _PROGRAM_CACHE[key]
    in_maps = make_in_maps(cfg, np.asarray(x), np.asarray(w_qkv),
                           np.asarray(b_qkv), np.asarray(w_dense))
    res = run_bass_kernel_spmd(nc, in_maps, list(range(cfg.n_cores)))
    return combine_outputs(cfg, res.results, np.asarray(b_dense))


# revision 48
# speedup vs baseline: 1.0839x; 1.0099x over previous
"""GPT-NeoX attention layer (B=2, S=2048, E=2048, H=16, partial RoPE 32/128)
as a Bass/Tile kernel for 8 Trainium2 NeuronCores.

Sharding: tensor-parallel across heads (2 heads per core, Megatron-style).
Each core computes QKV projection for its 2 heads over all tokens, applies
partial RoPE, runs causal attention, and produces a partial dense output
(contraction over its 256 columns of w_dense).  The 8 bf16 partial outputs
are summed on the host and the dense bias is added once on the host.

Everything on-device is bf16 (inputs pre-converted on the host); PSUM
accumulation stays fp32.  Key structure choices:

  qk_sb  [128, 4, SF]   Q^T/K^T per head (head dim on partitions) - scores
                        and y^T matmuls consume this directly.
  vnat   [128, SF/128, 256]  V in NATURAL [token, d] layout, produced in
                        phase 1 by x-stationary matmuls (x as lhsT), so no
                        PE transposes of V are ever needed.
  scores S^T = K^T.T @ Q^T in [sk, sq] blocks; exp on ScalarE (pipelined one
                        block behind the scores matmuls).
  softmax sums          via N=1 matmuls with the exp'd block as the
                        stationary operand (out [sq,1]): nearly free on PE,
                        instead of a 512-wide ones-matmul per block.
  normalize             reciprocal -> tiny PE transpose -> GPSIMD
                        partition_broadcast -> one DVE multiply per chunk.
  dense                 interleaved into later attention heads (fills the
                        tensor engine while ScalarE works through exp).
"""

import numpy as np
from contextlib import ExitStack

import concourse.bass as bass
import concourse.bacc as bacc
import concourse.mybir as mybir
import concourse.tile as tile
from concourse.masks import make_identity

AF = mybir.ActivationFunctionType
F32 = mybir.dt.float32
BF16 = mybir.dt.bfloat16

NEG_MASK = -1.0e9


class Cfg:
    def __init__(self, B=2, S=2048, E=2048, H=16, n_cores=8):
        self.B, self.S, self.E, self.H = B, S, E, H
        self.HS = 128                  # head size (one partition tile)
        self.ROT = 32                  # rotary dims
        self.n_cores = n_cores
        self.HPC = H // n_cores        # heads per core
        assert self.HPC == 2, "kernel assumes 2 heads per core"
        self.NQK = 2 * self.HPC        # q/k row tiles (h0q,h0k,h1q,h1k)
        self.VW = self.HPC * self.HS   # v natural width (d per core)
        self.RW = self.NQK * self.HS   # per-core q+k rows
        self.WCOLS = self.RW + self.VW
        self.SF = B * S
        self.KT = E // 128             # contraction tiles
        self.SC = 256                  # phase-1 token chunk
        self.NP1 = self.SF // self.SC
        self.G = self.SF // 4          # rope regroup width
        self.NQC = S // 512            # q chunks per (b, h)
        self.EO = E // 128             # dense output row tiles
        self.CT = self.HPC             # dense contraction tiles
        self.SCALE = 1.0 / np.sqrt(self.HS)
        assert S % 512 == 0 and E % 128 == 0 and self.SF % (4 * self.SC) == 0


class _Feeder:
    """FIFO of deferred dense micro-step generators, materialized from
    (b, scp) specs once `factory` is set and the feeder is enabled."""

    def __init__(self):
        self.specs = []
        self.gens = []
        self.factory = None
        self.enabled = False

    def push(self, spec):
        self.specs.append(spec)

    def _refill(self):
        if not self.gens and self.specs and self.factory:
            self.gens.append(self.factory(*self.specs.pop(0)))

    def step(self):
        if not self.enabled:
            return
        self._refill()
        while self.gens:
            try:
                next(self.gens[0])
                return
            except StopIteration:
                self.gens.pop(0)
                self._refill()

    def drain(self):
        assert self.enabled
        while True:
            self._refill()
            if not self.gens:
                return
            for _ in self.gens.pop(0):
                pass


class _Pump:
    """Steps an attention generator one j-block at a time; the generator
    yields an int (p1 chunks that must be emitted first) before each chunk
    and None per block."""

    def __init__(self, gen):
        self.gen = gen
        self.parked = None
        self.done = False

    def step(self, sc):
        if self.done:
            return False
        if self.parked is not None:
            if self.parked > sc:
                return False
            self.parked = None
        while True:
            try:
                v = next(self.gen)
            except StopIteration:
                self.done = True
                return False
            if v is None:
                return True
            if v > sc:
                self.parked = v
                return False


def build_program(cfg: Cfg, debug: bool = False) -> bass.Bass:
    B, S, E = cfg.B, cfg.S, cfg.E
    SF, KT, G = cfg.SF, cfg.KT, cfg.G
    SC, NQK, VW, RW = cfg.SC, cfg.NQK, cfg.VW, cfg.RW
    HPC, CT, EO = cfg.HPC, cfg.CT, cfg.EO
    NT = SF // 128                   # vnat token tiles

    nc = bacc.Bacc(None)
    xT = nc.dram_tensor("xT", [E, SF], BF16, kind="ExternalInput")
    wcat = nc.dram_tensor("wcat", [E, cfg.WCOLS], BF16, kind="ExternalInput")
    bqk = nc.dram_tensor("bqk", [RW], F32, kind="ExternalInput")
    vbbc = nc.dram_tensor("vbbc", [128, VW], F32, kind="ExternalInput")
    wdT = nc.dram_tensor("wdT", [VW, E], BF16, kind="ExternalInput")
    cosT = nc.dram_tensor("cosT", [32, SF], BF16, kind="ExternalInput")
    sinT = nc.dram_tensor("sinT", [32, SF], BF16, kind="ExternalInput")
    maskT = nc.dram_tensor("maskT", [128, 128], BF16, kind="ExternalInput")
    outT = nc.dram_tensor("outT", [E, SF], BF16, kind="ExternalOutput")

    with tile.TileContext(nc) as tc, ExitStack() as stk:
        consts = stk.enter_context(tc.tile_pool(name="consts", bufs=1))
        bigp = stk.enter_context(tc.tile_pool(name="big", bufs=1))
        qk_sb = bigp.tile([128, NQK, SF], BF16)
        vnat = bigp.tile([128, NT, VW], BF16)
        yT_sb = bigp.tile([128, CT, SF], BF16)

        # constants (tiles declared here; filled during phase-1 emission,
        # after the critical w/x DMA stream is issued)
        ident = consts.tile([128, 128], F32)
        identB = consts.tile([128, 128], BF16)
        ones_col = consts.tile([128, 1], BF16)
        mask_sb = consts.tile([128, 128], BF16)
        bqk_sb = consts.tile([128, NQK], F32)
        vb_sb = consts.tile([128, VW], F32)
        cos_sb = consts.tile([32, SF], BF16, tag="costab")
        sin_sb = consts.tile([32, SF], BF16, tag="sintab")
        wd_sb = consts.tile([128, CT, E], BF16, tag="wd")

        # RoPE: rotate_half is a partition swap within the 32 rot rows ->
        # DVE stream_shuffle (per-quadrant permutation) + elementwise
        # combine in [32, cols] layout, zero DMAs.  Each 1024-col slice is
        # emitted as soon as the phase-1 chunks covering it are done.
        SW = 1024
        rope_mask = [(i + 16) % 32 for i in range(32)]
        ropep = stk.enter_context(tc.tile_pool(name="rope", bufs=2))

        def rope_slice(i, sl):
            cs = slice(sl * SW, (sl + 1) * SW)
            blk = qk_sb[0:cfg.ROT, i, cs]
            sw = ropep.tile([32, SW], BF16, tag="swap", name="sw")
            nc.vector.stream_shuffle(sw, blk, rope_mask)
            with nc.allow_low_precision(reason="bf16 rope"):
                nc.vector.tensor_mul(sw, sw, sin_sb[:, cs])
                nc.vector.tensor_mul(blk, blk, cos_sb[:, cs])
                nc.vector.tensor_add(blk, blk, sw)

        # ---------------- Attention pools (live through phase 1) ----------
        feeder = _Feeder()
        ptp = stk.enter_context(tc.tile_pool(name="pstrip", bufs=6))
        npool = stk.enter_context(tc.tile_pool(name="norm", bufs=2))
        psA = stk.enter_context(tc.tile_pool(name="psA", bufs=2, space="PSUM"))
        psY = stk.enter_context(tc.tile_pool(name="psY", bufs=2, space="PSUM"))
        psS = stk.enter_context(tc.tile_pool(name="psS", bufs=1, space="PSUM"))
        LAG = 3   # j-blocks between scores+exp emission and yacc+sums

        def attention(b, hl, on_chain=None):
            """Generator: yields the p1-chunk prerequisite (int) before each
            q-chunk, then None after each emitted j-block."""
            scol = b * S
            q_t = qk_sb[:, 2 * hl, scol:scol + S]
            k_t = qk_sb[:, 2 * hl + 1, scol:scol + S]

            def emit_chain(c, psYt, psSt):
                recip = npool.tile([128, 4], BF16, tag="recip")
                with nc.allow_low_precision(reason="bf16 recip"):
                    nc.vector.reciprocal(recip, psSt[:, 0:4])
                # transpose each recip column to partition 0 ([1, 128])
                rps = psA.tile([128, 512], BF16, tag="A", name="rps")
                for g in range(4):
                    nc.tensor.matmul(
                        rps[0:1, 128 * g:128 * (g + 1)],
                        recip[:, g:g + 1], identB,
                        is_transpose=True, start=(g == 0), stop=(g == 3),
                        skip_group_check=True)
                rT = npool.tile([1, 512], F32, tag="rT")
                nc.vector.tensor_copy(rT, rps[0:1, 0:512])
                bc = npool.tile([128, 512], F32, tag="bc")
                for g in range(4):
                    nc.gpsimd.partition_broadcast(
                        bc[:, 128 * g:128 * (g + 1)],
                        rT[0:1, 128 * g:128 * (g + 1)])
                with nc.allow_low_precision(reason="bf16 y eviction"):
                    nc.vector.tensor_mul(
                        yT_sb[:, hl, scol + c * 512:scol + (c + 1) * 512],
                        psYt[:, 0:512], bc)
                if on_chain is not None:
                    on_chain(c)

            for c in range(cfg.NQC):
                # p1 chunks needed: data cols + one extra chunk so the rope
                # DVE work emitted at the slice boundary is already done
                sl_needed = (b * S + 512 * (c + 1) - 1) // SW
                yield (sl_needed + 1) * (SW // SC) + hl
                nj = 4 * (c + 1)
                psYt = psY.tile([128, 512], F32, tag="Y")
                psSt = psS.tile([128, 4], F32, tag="S")
                pend = []

                def emit_ys(j, pT, off, g0, psYt=psYt, psSt=psSt, c=c,
                            nj=nj):
                    nc.tensor.matmul(
                        psYt[:, off:512],
                        vnat[:, b * (S // 128) + j, 128 * hl:128 * (hl + 1)],
                        pT[:, off:512],
                        start=(j == 0), stop=(j == nj - 1),
                        skip_group_check=True)
                    for g in range(g0, 4):
                        # start only on the very first sums matmul of the
                        # chunk (bank-wide zero region); later columns
                        # accumulate onto pending-zero bytes.
                        nc.tensor.matmul(
                            psSt[:, g:g + 1],
                            pT[:, 128 * g:128 * (g + 1)], ones_col,
                            start=(j == 0 and g == 0),
                            stop=(j == nj - 1 and g == 3),
                            skip_group_check=True)

                for j in range(nj):
                    g0 = max(0, j - 4 * c)
                    off = 128 * g0
                    ps = psA.tile([128, 512], F32, tag="A")
                    nc.tensor.matmul(
                        ps[:, off:512],
                        k_t[:, 128 * j:128 * (j + 1)],
                        q_t[:, c * 512 + off:(c + 1) * 512],
                        start=True, stop=True, skip_group_check=True)
                    pT = ptp.tile([128, 512], BF16, tag=f"p{hl}", name="pT")
                    nc.scalar.activation(
                        pT[:, off:512], ps[:, off:512], AF.Exp,
                        scale=cfg.SCALE)
                    if j >= 4 * c:
                        # causal mask as a cheap post-exp 0/1 multiply
                        with nc.allow_low_precision(reason="bf16 mask"):
                            nc.vector.tensor_mul(
                                pT[:, off:off + 128],
                                pT[:, off:off + 128], mask_sb)
                    if len(pend) >= LAG:
                        emit_ys(*pend.pop(0))
                    feeder.step()
                    pend.append((j, pT, off, g0))
                    yield None
                while pend:
                    emit_ys(*pend.pop(0))
                feeder.step()
                # chunk-end chain emitted atomically (the single psS bank
                # must be read here before another head's sums start)
                emit_chain(c, psYt, psSt)

        def make_pump(b, hl):
            hook = None
            if hl == HPC - 1:
                def hook(c, b=b):
                    if c % 2 == 1:
                        feeder.push((b, (c - 1) // 2))
            return _Pump(attention(b, hl, on_chain=hook))

        pumps_b0 = [make_pump(0, hl) for hl in range(HPC)]
        slot_budget = [0]
        active = [0]

        def inject(sc):
            # Called at points inside phase-1 emission; injects one ready
            # attention j-block into the PE stream.  Sticky generator choice:
            # a generator only loses its turn at a chunk boundary, so pool
            # tiles shared across generators (psS, psY) stay chunk-atomic.
            if slot_budget[0] <= 0:
                return
            n = len(pumps_b0)
            for off in range(n):
                p = pumps_b0[(active[0] + off) % n]
                if p.step(sc):
                    active[0] = (active[0] + off) % n
                    slot_budget[0] -= 1
                    return

        # ---------------- Phase 1: QKV projection ------------------------
        with tc.tile_pool(name="wq", bufs=1) as wp, \
             tc.tile_pool(name="xs", bufs=2) as xp, \
             tc.tile_pool(name="ps1", bufs=1, space="PSUM") as pp1:
            w_sb = wp.tile([128, KT, cfg.WCOLS], BF16)
            w_view = wcat.rearrange("(kt p) r -> p kt r", p=128)
            x_view = xT.rearrange("(kt p) s -> p kt s", p=128)

            # interleave per-kt w loads with quarters of the first x chunk
            xt0 = xp.tile([128, KT, SC], BF16, tag="xt")
            ktg = max(1, KT // 4)
            for q0 in range(0, KT, ktg):
                q1 = min(q0 + ktg, KT)
                for kt in range(q0, q1):
                    nc.sync.dma_start(out=w_sb[:, kt, :], in_=w_view[:, kt, :])
                nc.sync.dma_start(out=xt0[:, q0:q1, :],
                                  in_=x_view[:, q0:q1, 0:SC])
            xt1 = xp.tile([128, KT, SC], BF16, tag="xt")
            nc.sync.dma_start(out=xt1[:, :, :], in_=x_view[:, :, SC:2 * SC])

            # constants (after the critical w/x stream)
            make_identity(nc, ident)
            with nc.allow_low_precision(reason="bf16 identity"):
                nc.vector.tensor_copy(identB, ident)
            nc.vector.memset(ones_col, 1.0)
            nc.sync.dma_start(out=mask_sb, in_=maskT[:, :])
            nc.sync.dma_start(out=bqk_sb,
                              in_=bqk.rearrange("(rt p) -> p rt", p=128))
            nc.sync.dma_start(out=vb_sb, in_=vbbc[:, :])
            nc.sync.dma_start(out=cos_sb, in_=cosT[:, :])
            nc.sync.dma_start(out=sin_sb, in_=sinT[:, :])
            nc.sync.dma_start(
                out=wd_sb[:, :, :],
                in_=wdT.rearrange("(ct p) e -> p ct e", p=128))

            ntile = SC // 128   # v token sub-tiles per chunk (=2)
            # group-major chunks: 6 sequential accumulation groups cycling
            # 3 single-buffered banks (paired groups share a bank via the
            # pending-zero trick), evicted inline as each group finishes.
            for sc in range(cfg.NP1):
                slot_budget[0] = 7
                if sc == 0:
                    xt = xt0
                elif sc == 1:
                    xt = xt1
                else:
                    xt = xp.tile([128, KT, SC], BF16, tag="xt")
                    nc.sync.dma_start(
                        out=xt[:, :, :],
                        in_=x_view[:, :, sc * SC:(sc + 1) * SC])
                def p1_mm(pt, grp, half, kt):
                    reg = pt[:, 256 * half:256 * half + 256]
                    fl, ll = (kt == 0), (kt == KT - 1)
                    if grp < NQK:
                        nc.tensor.matmul(
                            reg,
                            w_sb[:, kt, 128 * grp:128 * (grp + 1)],
                            xt[:, kt, :],
                            start=fl and half == 0, stop=ll and half == 1,
                            skip_group_check=True)
                    else:
                        t = grp - NQK
                        nc.tensor.matmul(
                            reg,
                            xt[:, kt, 128 * t:128 * (t + 1)],
                            w_sb[:, kt, RW:RW + VW],
                            start=fl and half == 0, stop=ll and half == 1,
                            skip_group_check=True)

                def p1_evict(pt, grp, half):
                    reg = pt[:, 256 * half:256 * half + 256]
                    if grp < NQK:
                        nc.scalar.activation(
                            qk_sb[:, grp, sc * SC:(sc + 1) * SC], reg,
                            AF.Identity, bias=bqk_sb[:, grp:grp + 1])
                    else:
                        t = grp - NQK
                        with nc.allow_low_precision(reason="bf16 v"):
                            nc.vector.tensor_add(
                                vnat[:, sc * ntile + t, :], reg, vb_sb)

                if sc < 2:
                    # kt-outer while the w stream is still arriving: consume
                    # each w[kt] across all 6 groups as soon as it lands
                    pts = [pp1.tile([128, 512], F32, tag=f"p1{p}",
                                    name=f"p1ps{p}") for p in range(3)]
                    for kt in range(KT):
                        for grp in range(6):
                            p1_mm(pts[grp // 2], grp, grp % 2, kt)
                    for grp in range(6):
                        p1_evict(pts[grp // 2], grp, grp % 2)
                else:
                    # group-major: 6 sequential groups over 3 single-buffered
                    # banks, each evicted inline as it finishes
                    for pair in range(3):
                        pt = pp1.tile([128, 512], F32, tag=f"p1{pair}",
                                      name=f"p1ps{pair}")
                        for half in range(2):
                            for kt in range(KT):
                                p1_mm(pt, 2 * pair + half, half, kt)
                                if kt % 4 == 3:
                                    inject(sc - 1)
                            p1_evict(pt, 2 * pair + half, half)
                if (sc + 1) % (SW // SC) == 0:
                    for i in range(NQK):
                        rope_slice(i, sc // (SW // SC))

        # ---------------- Tail: remaining attention + dense ----------------
        with tc.tile_pool(name="outsb", bufs=4) as op, \
             tc.tile_pool(name="psD", bufs=2, space="PSUM") as psD:

            def dense_steps(b, scp):
                # one (eo) output row-tile over two 512-token col chunks
                for eo in range(EO):
                    ot = op.tile([128, 1024], BF16, tag="out")
                    for t in range(2):
                        col = b * S + (2 * scp + t) * 512
                        pd = psD.tile([128, 512], F32, tag="D")
                        for ct in range(CT):
                            nc.tensor.matmul(
                                pd,
                                wd_sb[:, ct, 128 * eo:128 * (eo + 1)],
                                yT_sb[:, ct, col:col + 512],
                                start=(ct == 0), stop=(ct == CT - 1),
                                skip_group_check=True)
                        yield
                        with nc.allow_low_precision(reason="bf16 out"):
                            if (eo + t) % 2 == 0:
                                nc.vector.tensor_copy(
                                    ot[:, 512 * t:512 * (t + 1)], pd)
                            else:
                                nc.scalar.activation(
                                    ot[:, 512 * t:512 * (t + 1)], pd, AF.Copy)
                    nc.sync.dma_start(
                        out=outT[128 * eo:128 * (eo + 1),
                                 b * S + scp * 1024:b * S + (scp + 1) * 1024],
                        in_=ot)
                    yield

            feeder.factory = dense_steps
            feeder.enabled = True
            BIG = 10 ** 9
            while any(p.step(BIG) for p in pumps_b0):
                pass
            for bb in range(1, B):
                pumps = [make_pump(bb, hl) for hl in range(HPC)]
                for p in pumps:
                    while p.step(BIG):
                        pass
            feeder.drain()

        if debug:
            dqk = nc.dram_tensor("dbg_qk", [128, NQK, SF], BF16,
                                 kind="ExternalOutput")
            dv = nc.dram_tensor("dbg_v", [128, NT, VW], BF16,
                                kind="ExternalOutput")
            dy = nc.dram_tensor("dbg_y", [128, CT, SF], BF16,
                                kind="ExternalOutput")
            nc.sync.dma_start(out=dqk[:, :, :], in_=qk_sb[:, :, :])
            nc.sync.dma_start(out=dv[:, :, :], in_=vnat[:, :, :])
            nc.sync.dma_start(out=dy[:, :, :], in_=yT_sb[:, :, :])

    nc.finalize()
    return nc


# ---------------------------------------------------------------------------
# Host-side input preparation / sharding
# ---------------------------------------------------------------------------

def _bf16(a):
    import ml_dtypes
    return np.ascontiguousarray(a, np.float32).astype(ml_dtypes.bfloat16)


def _rope_tables(cfg: Cfg):
    inv_freq = 1.0 / (10000.0 ** (np.arange(0, cfg.ROT, 2, dtype=np.float64)
                                  / cfg.ROT))
    t = np.arange(cfg.S, dtype=np.float64)
    freqs = np.outer(t, inv_freq)                       # [S, 16]
    emb = np.concatenate([freqs, freqs], axis=-1)       # [S, 32]
    cos = np.cos(emb).T.astype(np.float32)              # [32, S]
    sin = np.sin(emb).T.astype(np.float32)
    cosF = np.tile(cos, (1, cfg.B))                     # [32, SF]
    sinF = np.tile(sin, (1, cfg.B))
    sinF[:cfg.ROT // 2] *= -1.0                         # fold rotate_half sign
    return _bf16(cosF), _bf16(sinF)


def make_in_maps(cfg: Cfg, x, w_qkv, b_qkv, w_dense):
    HS, HPC = cfg.HS, cfg.HPC
    xTb = _bf16(np.ascontiguousarray(
        np.asarray(x, np.float32).reshape(cfg.SF, cfg.E).T))
    cos128, sin128s = _rope_tables(cfg)
    p = np.arange(128)[:, None]
    f = np.arange(128)[None, :]
    maskT = _bf16(np.where(p <= f, 1.0, 0.0))   # post-exp 0/1 causal mask
    in_maps = []
    for i in range(cfg.n_cores):
        heads = [HPC * i + h for h in range(HPC)]
        qk_rows = np.concatenate(
            [np.arange(h * 3 * HS + qk * HS, h * 3 * HS + (qk + 1) * HS)
             for h in heads for qk in range(2)])
        v_rows = np.concatenate(
            [np.arange(h * 3 * HS + 2 * HS, h * 3 * HS + 3 * HS)
             for h in heads])
        wcat = np.concatenate(
            [np.asarray(w_qkv, np.float32)[qk_rows, :].T,
             np.asarray(w_qkv, np.float32)[v_rows, :].T], axis=1)
        cols = slice(i * cfg.VW, (i + 1) * cfg.VW)
        in_maps.append({
            "xT": xTb,
            "wcat": _bf16(wcat),
            "bqk": np.ascontiguousarray(
                np.asarray(b_qkv, np.float32)[qk_rows]),
            "vbbc": np.ascontiguousarray(np.tile(
                np.asarray(b_qkv, np.float32)[v_rows][None, :], (128, 1))),
            "wdT": _bf16(np.asarray(w_dense, np.float32)[:, cols].T),
            "cosT": cos128,
            "sinT": sin128s,
            "maskT": maskT,
        })
    return in_maps


def combine_outputs(cfg: Cfg, results, b_dense):
    acc = np.zeros((cfg.E, cfg.SF), dtype=np.float64)
    for r in results:
        acc += np.asarray(r["outT"]).astype(np.float64)
    out = acc.T.reshape(cfg.B, cfg.S, cfg.E) + \
        np.asarray(b_dense, np.float64)
    return out.astype(np.float32)


_PROGRAM_CACHE = {}


def kernel(x, w_qkv, b_qkv, w_dense, b_dense):
    from concourse.bass_utils import run_bass_kernel_spmd

    cfg = Cfg()
    key = "full"
    if key not in _PROGRAM_CACHE:
        _PROGRAM_CACHE[key] = build_program(cfg)
    nc = _PROGRAM_CACHE[key]
    in_maps = make_in_maps(cfg, np.asarray(x), np.asarray(w_qkv),
                           np.asarray(b_qkv), np.asarray(w_dense))
    res = run_bass_kernel_spmd(nc, in_maps, list(range(cfg.n_cores)))
    return combine_outputs(cfg, res.results, np.asarray(b_dense))


# revision 49
# speedup vs baseline: 1.1042x; 1.0187x over previous
"""GPT-NeoX attention layer (B=2, S=2048, E=2048, H=16, partial RoPE 32/128)
as a Bass/Tile kernel for 8 Trainium2 NeuronCores.

Sharding: tensor-parallel across heads (2 heads per core, Megatron-style).
Each core computes QKV projection for its 2 heads over all tokens, applies
partial RoPE, runs causal attention, and produces a partial dense output
(contraction over its 256 columns of w_dense).  The 8 bf16 partial outputs
are summed on the host and the dense bias is added once on the host.

Everything on-device is bf16 (inputs pre-converted on the host); PSUM
accumulation stays fp32.  Key structure choices:

  qk_sb  [128, 4, SF]   Q^T/K^T per head (head dim on partitions) - scores
                        and y^T matmuls consume this directly.
  vnat   [128, SF/128, 256]  V in NATURAL [token, d] layout, produced in
                        phase 1 by x-stationary matmuls (x as lhsT), so no
                        PE transposes of V are ever needed.
  scores S^T = K^T.T @ Q^T in [sk, sq] blocks; exp on ScalarE (pipelined one
                        block behind the scores matmuls).
  softmax sums          via N=1 matmuls with the exp'd block as the
                        stationary operand (out [sq,1]): nearly free on PE,
                        instead of a 512-wide ones-matmul per block.
  normalize             reciprocal -> tiny PE transpose -> GPSIMD
                        partition_broadcast -> one DVE multiply per chunk.
  dense                 interleaved into later attention heads (fills the
                        tensor engine while ScalarE works through exp).
"""

import numpy as np
from contextlib import ExitStack

import concourse.bass as bass
import concourse.bacc as bacc
import concourse.mybir as mybir
import concourse.tile as tile
from concourse.masks import make_identity

AF = mybir.ActivationFunctionType
F32 = mybir.dt.float32
BF16 = mybir.dt.bfloat16

NEG_MASK = -1.0e9


class Cfg:
    def __init__(self, B=2, S=2048, E=2048, H=16, n_cores=8):
        self.B, self.S, self.E, self.H = B, S, E, H
        self.HS = 128                  # head size (one partition tile)
        self.ROT = 32                  # rotary dims
        self.n_cores = n_cores
        self.HPC = H // n_cores        # heads per core
        assert self.HPC == 2, "kernel assumes 2 heads per core"
        self.NQK = 2 * self.HPC        # q/k row tiles (h0q,h0k,h1q,h1k)
        self.VW = self.HPC * self.HS   # v natural width (d per core)
        self.RW = self.NQK * self.HS   # per-core q+k rows
        self.WCOLS = self.RW + self.VW
        self.SF = B * S
        self.KT = E // 128             # contraction tiles
        self.SC = 256                  # phase-1 token chunk
        self.NP1 = self.SF // self.SC
        self.G = self.SF // 4          # rope regroup width
        self.NQC = S // 512            # q chunks per (b, h)
        self.EO = E // 128             # dense output row tiles
        self.CT = self.HPC             # dense contraction tiles
        self.SCALE = 1.0 / np.sqrt(self.HS)
        assert S % 512 == 0 and E % 128 == 0 and self.SF % (4 * self.SC) == 0


class _Feeder:
    """FIFO of deferred dense micro-step generators, materialized from
    (b, scp) specs once `factory` is set and the feeder is enabled."""

    def __init__(self):
        self.specs = []
        self.gens = []
        self.factory = None
        self.enabled = False

    def push(self, spec):
        self.specs.append(spec)

    def _refill(self):
        if not self.gens and self.specs and self.factory:
            self.gens.append(self.factory(*self.specs.pop(0)))

    def step(self):
        if not self.enabled:
            return
        self._refill()
        while self.gens:
            try:
                next(self.gens[0])
                return
            except StopIteration:
                self.gens.pop(0)
                self._refill()

    def drain(self):
        assert self.enabled
        while True:
            self._refill()
            if not self.gens:
                return
            for _ in self.gens.pop(0):
                pass


class _Pump:
    """Steps an attention generator one j-block at a time; the generator
    yields an int (p1 chunks that must be emitted first) before each chunk
    and None per block."""

    def __init__(self, gen):
        self.gen = gen
        self.parked = None
        self.done = False

    def step(self, sc):
        if self.done:
            return False
        if self.parked is not None:
            if self.parked > sc:
                return False
            self.parked = None
        while True:
            try:
                v = next(self.gen)
            except StopIteration:
                self.done = True
                return False
            if v is None:
                return True
            if v > sc:
                self.parked = v
                return False


def build_program(cfg: Cfg, debug: bool = False) -> bass.Bass:
    B, S, E = cfg.B, cfg.S, cfg.E
    SF, KT, G = cfg.SF, cfg.KT, cfg.G
    SC, NQK, VW, RW = cfg.SC, cfg.NQK, cfg.VW, cfg.RW
    HPC, CT, EO = cfg.HPC, cfg.CT, cfg.EO
    NT = SF // 128                   # vnat token tiles

    nc = bacc.Bacc(None)
    xT = nc.dram_tensor("xT", [E, SF], BF16, kind="ExternalInput")
    wcat = nc.dram_tensor("wcat", [E, cfg.WCOLS], BF16, kind="ExternalInput")
    bqk = nc.dram_tensor("bqk", [RW], F32, kind="ExternalInput")
    vbbc = nc.dram_tensor("vbbc", [128, VW], F32, kind="ExternalInput")
    wdT = nc.dram_tensor("wdT", [VW, E], BF16, kind="ExternalInput")
    cosT = nc.dram_tensor("cosT", [32, SF], BF16, kind="ExternalInput")
    sinT = nc.dram_tensor("sinT", [32, SF], BF16, kind="ExternalInput")
    maskT = nc.dram_tensor("maskT", [128, 128], BF16, kind="ExternalInput")
    outT = nc.dram_tensor("outT", [E, SF], BF16, kind="ExternalOutput")

    with tile.TileContext(nc) as tc, ExitStack() as stk:
        consts = stk.enter_context(tc.tile_pool(name="consts", bufs=1))
        bigp = stk.enter_context(tc.tile_pool(name="big", bufs=1))
        qk_sb = bigp.tile([128, NQK, SF], BF16)
        vnat = bigp.tile([128, NT, VW], BF16)
        yT_sb = bigp.tile([128, CT, SF], BF16)

        # constants (tiles declared here; filled during phase-1 emission,
        # after the critical w/x DMA stream is issued)
        ident = consts.tile([128, 128], F32)
        identB = consts.tile([128, 128], BF16)
        ones_col = consts.tile([128, 1], BF16)
        mask_sb = consts.tile([128, 128], BF16)
        bqk_sb = consts.tile([128, NQK], F32)
        vb_sb = consts.tile([128, VW], F32)
        cos_sb = consts.tile([32, SF], BF16, tag="costab")
        sin_sb = consts.tile([32, SF], BF16, tag="sintab")
        wd_sb = consts.tile([128, CT, E], BF16, tag="wd")

        # RoPE: rotate_half is a partition swap within the 32 rot rows ->
        # DVE stream_shuffle (per-quadrant permutation) + elementwise
        # combine in [32, cols] layout, zero DMAs.  Each 1024-col slice is
        # emitted as soon as the phase-1 chunks covering it are done.
        SW = 1024
        rope_mask = [(i + 16) % 32 for i in range(32)]
        ropep = stk.enter_context(tc.tile_pool(name="rope", bufs=2))

        def rope_slice(i, sl):
            cs = slice(sl * SW, (sl + 1) * SW)
            blk = qk_sb[0:cfg.ROT, i, cs]
            sw = ropep.tile([32, SW], BF16, tag="swap", name="sw")
            nc.vector.stream_shuffle(sw, blk, rope_mask)
            with nc.allow_low_precision(reason="bf16 rope"):
                nc.vector.tensor_mul(sw, sw, sin_sb[:, cs])
                nc.vector.tensor_mul(blk, blk, cos_sb[:, cs])
                nc.vector.tensor_add(blk, blk, sw)

        # ---------------- Attention pools (live through phase 1) ----------
        feeder = _Feeder()
        ptp = stk.enter_context(tc.tile_pool(name="pstrip", bufs=6))
        npool = stk.enter_context(tc.tile_pool(name="norm", bufs=2))
        psA = stk.enter_context(tc.tile_pool(name="psA", bufs=2, space="PSUM"))
        psY = stk.enter_context(tc.tile_pool(name="psY", bufs=2, space="PSUM"))
        psS = stk.enter_context(tc.tile_pool(name="psS", bufs=1, space="PSUM"))
        LAG = 3   # j-blocks between scores+exp emission and yacc+sums

        def attention(b, hl, on_chain=None):
            """Generator: yields the p1-chunk prerequisite (int) before each
            q-chunk, then None after each emitted j-block."""
            scol = b * S
            q_t = qk_sb[:, 2 * hl, scol:scol + S]
            k_t = qk_sb[:, 2 * hl + 1, scol:scol + S]

            def emit_chain(c, psYt, psSt):
                recip = npool.tile([128, 4], BF16, tag="recip")
                with nc.allow_low_precision(reason="bf16 recip"):
                    nc.vector.reciprocal(recip, psSt[:, 0:4])
                # transpose each recip column to partition 0 ([1, 128])
                rps = psA.tile([128, 512], BF16, tag="A", name="rps")
                for g in range(4):
                    nc.tensor.matmul(
                        rps[0:1, 128 * g:128 * (g + 1)],
                        recip[:, g:g + 1], identB,
                        is_transpose=True, start=(g == 0), stop=(g == 3),
                        skip_group_check=True)
                rT = npool.tile([1, 512], F32, tag="rT")
                nc.vector.tensor_copy(rT, rps[0:1, 0:512])
                bc = npool.tile([128, 512], F32, tag="bc")
                for g in range(4):
                    nc.gpsimd.partition_broadcast(
                        bc[:, 128 * g:128 * (g + 1)],
                        rT[0:1, 128 * g:128 * (g + 1)])
                with nc.allow_low_precision(reason="bf16 y eviction"):
                    nc.vector.tensor_mul(
                        yT_sb[:, hl, scol + c * 512:scol + (c + 1) * 512],
                        psYt[:, 0:512], bc)
                if on_chain is not None:
                    on_chain(c)

            for c in range(cfg.NQC):
                # p1 chunks needed: data cols + one extra chunk so the rope
                # DVE work emitted at the slice boundary is already done
                sl_needed = (b * S + 512 * (c + 1) - 1) // SW
                yield (sl_needed + 1) * (SW // SC) + hl
                nj = 4 * (c + 1)
                psYt = psY.tile([128, 512], F32, tag="Y")
                psSt = psS.tile([128, 4], F32, tag="S")
                pend = []

                def emit_ys(j, pT, off, g0, psYt=psYt, psSt=psSt, c=c,
                            nj=nj):
                    nc.tensor.matmul(
                        psYt[:, off:512],
                        vnat[:, b * (S // 128) + j, 128 * hl:128 * (hl + 1)],
                        pT[:, off:512],
                        start=(j == 0), stop=(j == nj - 1),
                        skip_group_check=True)
                    for g in range(g0, 4):
                        # start only on the very first sums matmul of the
                        # chunk (bank-wide zero region); later columns
                        # accumulate onto pending-zero bytes.
                        nc.tensor.matmul(
                            psSt[:, g:g + 1],
                            pT[:, 128 * g:128 * (g + 1)], ones_col,
                            start=(j == 0 and g == 0),
                            stop=(j == nj - 1 and g == 3),
                            skip_group_check=True)

                for j in range(nj):
                    g0 = max(0, j - 4 * c)
                    off = 128 * g0
                    ps = psA.tile([128, 512], F32, tag="A")
                    nc.tensor.matmul(
                        ps[:, off:512],
                        k_t[:, 128 * j:128 * (j + 1)],
                        q_t[:, c * 512 + off:(c + 1) * 512],
                        start=True, stop=True, skip_group_check=True)
                    pT = ptp.tile([128, 512], BF16, tag=f"p{hl}", name="pT")
                    nc.scalar.activation(
                        pT[:, off:512], ps[:, off:512], AF.Exp,
                        scale=cfg.SCALE)
                    if j >= 4 * c:
                        # causal mask as a cheap post-exp 0/1 multiply
                        with nc.allow_low_precision(reason="bf16 mask"):
                            nc.vector.tensor_mul(
                                pT[:, off:off + 128],
                                pT[:, off:off + 128], mask_sb)
                    if len(pend) >= LAG:
                        emit_ys(*pend.pop(0))
                    feeder.step()
                    feeder.step()
                    pend.append((j, pT, off, g0))
                    yield None
                while pend:
                    emit_ys(*pend.pop(0))
                feeder.step()
                # chunk-end chain emitted atomically (the single psS bank
                # must be read here before another head's sums start)
                emit_chain(c, psYt, psSt)

        def make_pump(b, hl):
            hook = None
            if hl == HPC - 1:
                def hook(c, b=b):
                    if c % 2 == 1:
                        feeder.push((b, (c - 1) // 2))
            return _Pump(attention(b, hl, on_chain=hook))

        pumps_b0 = [make_pump(0, hl) for hl in range(HPC)]
        slot_budget = [0]
        active = [0]

        def inject(sc):
            # Called at points inside phase-1 emission; injects one ready
            # attention j-block into the PE stream.  Sticky generator choice:
            # a generator only loses its turn at a chunk boundary, so pool
            # tiles shared across generators (psS, psY) stay chunk-atomic.
            if slot_budget[0] <= 0:
                return
            n = len(pumps_b0)
            for off in range(n):
                p = pumps_b0[(active[0] + off) % n]
                if p.step(sc):
                    active[0] = (active[0] + off) % n
                    slot_budget[0] -= 1
                    return

        # ---------------- Phase 1: QKV projection ------------------------
        with tc.tile_pool(name="wq", bufs=1) as wp, \
             tc.tile_pool(name="xs", bufs=2) as xp, \
             tc.tile_pool(name="ps1", bufs=1, space="PSUM") as pp1:
            w_sb = wp.tile([128, KT, cfg.WCOLS], BF16)
            w_view = wcat.rearrange("(kt p) r -> p kt r", p=128)
            x_view = xT.rearrange("(kt p) s -> p kt s", p=128)

            # interleave per-kt w loads with quarters of the first x chunk
            xt0 = xp.tile([128, KT, SC], BF16, tag="xt")
            ktg = max(1, KT // 4)
            for q0 in range(0, KT, ktg):
                q1 = min(q0 + ktg, KT)
                for kt in range(q0, q1):
                    nc.sync.dma_start(out=w_sb[:, kt, :], in_=w_view[:, kt, :])
                nc.sync.dma_start(out=xt0[:, q0:q1, :],
                                  in_=x_view[:, q0:q1, 0:SC])
            xt1 = xp.tile([128, KT, SC], BF16, tag="xt")
            nc.sync.dma_start(out=xt1[:, :, :], in_=x_view[:, :, SC:2 * SC])

            # constants (after the critical w/x stream)
            make_identity(nc, ident)
            with nc.allow_low_precision(reason="bf16 identity"):
                nc.vector.tensor_copy(identB, ident)
            nc.vector.memset(ones_col, 1.0)
            nc.sync.dma_start(out=mask_sb, in_=maskT[:, :])
            nc.sync.dma_start(out=bqk_sb,
                              in_=bqk.rearrange("(rt p) -> p rt", p=128))
            nc.sync.dma_start(out=vb_sb, in_=vbbc[:, :])
            nc.sync.dma_start(out=cos_sb, in_=cosT[:, :])
            nc.sync.dma_start(out=sin_sb, in_=sinT[:, :])
            nc.sync.dma_start(
                out=wd_sb[:, :, :],
                in_=wdT.rearrange("(ct p) e -> p ct e", p=128))

            ntile = SC // 128   # v token sub-tiles per chunk (=2)
            # group-major chunks: 6 sequential accumulation groups cycling
            # 3 single-buffered banks (paired groups share a bank via the
            # pending-zero trick), evicted inline as each group finishes.
            for sc in range(cfg.NP1):
                slot_budget[0] = 7
                if sc == 0:
                    xt = xt0
                elif sc == 1:
                    xt = xt1
                else:
                    xt = xp.tile([128, KT, SC], BF16, tag="xt")
                    nc.sync.dma_start(
                        out=xt[:, :, :],
                        in_=x_view[:, :, sc * SC:(sc + 1) * SC])
                def p1_mm(pt, grp, half, kt):
                    reg = pt[:, 256 * half:256 * half + 256]
                    fl, ll = (kt == 0), (kt == KT - 1)
                    if grp < NQK:
                        nc.tensor.matmul(
                            reg,
                            w_sb[:, kt, 128 * grp:128 * (grp + 1)],
                            xt[:, kt, :],
                            start=fl and half == 0, stop=ll and half == 1,
                            skip_group_check=True)
                    else:
                        t = grp - NQK
                        nc.tensor.matmul(
                            reg,
                            xt[:, kt, 128 * t:128 * (t + 1)],
                            w_sb[:, kt, RW:RW + VW],
                            start=fl and half == 0, stop=ll and half == 1,
                            skip_group_check=True)

                def p1_evict(pt, grp, half):
                    reg = pt[:, 256 * half:256 * half + 256]
                    if grp < NQK:
                        nc.scalar.activation(
                            qk_sb[:, grp, sc * SC:(sc + 1) * SC], reg,
                            AF.Identity, bias=bqk_sb[:, grp:grp + 1])
                    else:
                        t = grp - NQK
                        with nc.allow_low_precision(reason="bf16 v"):
                            nc.vector.tensor_add(
                                vnat[:, sc * ntile + t, :], reg, vb_sb)

                if sc < 2:
                    # kt-outer while the w stream is still arriving: consume
                    # each w[kt] across all 6 groups as soon as it lands
                    pts = [pp1.tile([128, 512], F32, tag=f"p1{p}",
                                    name=f"p1ps{p}") for p in range(3)]
                    for kt in range(KT):
                        for grp in range(6):
                            p1_mm(pts[grp // 2], grp, grp % 2, kt)
                    for grp in range(6):
                        p1_evict(pts[grp // 2], grp, grp % 2)
                else:
                    # group-major: 6 sequential groups over 3 single-buffered
                    # banks, each evicted inline as it finishes
                    for pair in range(3):
                        pt = pp1.tile([128, 512], F32, tag=f"p1{pair}",
                                      name=f"p1ps{pair}")
                        for half in range(2):
                            for kt in range(KT):
                                p1_mm(pt, 2 * pair + half, half, kt)
                                if kt % 4 == 3:
                                    inject(sc - 1)
                            p1_evict(pt, 2 * pair + half, half)
                if (sc + 1) % (SW // SC) == 0:
                    for i in range(NQK):
                        rope_slice(i, sc // (SW // SC))

        # ---------------- Tail: remaining attention + dense ----------------
        with tc.tile_pool(name="outsb", bufs=4) as op, \
             tc.tile_pool(name="psD", bufs=2, space="PSUM") as psD:

            def dense_steps(b, scp):
                # one (eo) output row-tile over two 512-token col chunks
                for eo in range(EO):
                    ot = op.tile([128, 1024], BF16, tag="out")
                    for t in range(2):
                        col = b * S + (2 * scp + t) * 512
                        pd = psD.tile([128, 512], F32, tag="D")
                        for ct in range(CT):
                            nc.tensor.matmul(
                                pd,
                                wd_sb[:, ct, 128 * eo:128 * (eo + 1)],
                                yT_sb[:, ct, col:col + 512],
                                start=(ct == 0), stop=(ct == CT - 1),
                                skip_group_check=True)
                        yield
                        with nc.allow_low_precision(reason="bf16 out"):
                            if (eo + t) % 2 == 0:
                                nc.vector.tensor_copy(
                                    ot[:, 512 * t:512 * (t + 1)], pd)
                            else:
                                nc.scalar.activation(
                                    ot[:, 512 * t:512 * (t + 1)], pd, AF.Copy)
                    nc.sync.dma_start(
                        out=outT[128 * eo:128 * (eo + 1),
                                 b * S + scp * 1024:b * S + (scp + 1) * 1024],
                        in_=ot)
                    yield

            feeder.factory = dense_steps
            feeder.enabled = True
            BIG = 10 ** 9
            while any(p.step(BIG) for p in pumps_b0):
                pass
            for bb in range(1, B):
                pumps = [make_pump(bb, hl) for hl in range(HPC)]
                for p in pumps:
                    while p.step(BIG):
                        pass
            feeder.drain()

        if debug:
            dqk = nc.dram_tensor("dbg_qk", [128, NQK, SF], BF16,
                                 kind="ExternalOutput")
            dv = nc.dram_tensor("dbg_v", [128, NT, VW], BF16,
                                kind="ExternalOutput")
            dy = nc.dram_tensor("dbg_y", [128, CT, SF], BF16,
                                kind="ExternalOutput")
            nc.sync.dma_start(out=dqk[:, :, :], in_=qk_sb[:, :, :])
            nc.sync.dma_start(out=dv[:, :, :], in_=vnat[:, :, :])
            nc.sync.dma_start(out=dy[:, :, :], in_=yT_sb[:, :, :])

    nc.finalize()
    return nc


# ---------------------------------------------------------------------------
# Host-side input preparation / sharding
# ---------------------------------------------------------------------------

def _bf16(a):
    import ml_dtypes
    return np.ascontiguousarray(a, np.float32).astype(ml_dtypes.bfloat16)


def _rope_tables(cfg: Cfg):
    inv_freq = 1.0 / (10000.0 ** (np.arange(0, cfg.ROT, 2, dtype=np.float64)
                                  / cfg.ROT))
    t = np.arange(cfg.S, dtype=np.float64)
    freqs = np.outer(t, inv_freq)                       # [S, 16]
    emb = np.concatenate([freqs, freqs], axis=-1)       # [S, 32]
    cos = np.cos(emb).T.astype(np.float32)              # [32, S]
    sin = np.sin(emb).T.astype(np.float32)
    cosF = np.tile(cos, (1, cfg.B))                     # [32, SF]
    sinF = np.tile(sin, (1, cfg.B))
    sinF[:cfg.ROT // 2] *= -1.0                         # fold rotate_half sign
    return _bf16(cosF), _bf16(sinF)


def make_in_maps(cfg: Cfg, x, w_qkv, b_qkv, w_dense):
    HS, HPC = cfg.HS, cfg.HPC
    xTb = _bf16(np.ascontiguousarray(
        np.asarray(x, np.float32).reshape(cfg.SF, cfg.E).T))
    cos128, sin128s = _rope_tables(cfg)
    p = np.arange(128)[:, None]
    f = np.arange(128)[None, :]
    maskT = _bf16(np.where(p <= f, 1.0, 0.0))   # post-exp 0/1 causal mask
    in_maps = []
    for i in range(cfg.n_cores):
        heads = [HPC * i + h for h in range(HPC)]
        qk_rows = np.concatenate(
            [np.arange(h * 3 * HS + qk * HS, h * 3 * HS + (qk + 1) * HS)
             for h in heads for qk in range(2)])
        v_rows = np.concatenate(
            [np.arange(h * 3 * HS + 2 * HS, h * 3 * HS + 3 * HS)
             for h in heads])
        wcat = np.concatenate(
            [np.asarray(w_qkv, np.float32)[qk_rows, :].T,
             np.asarray(w_qkv, np.float32)[v_rows, :].T], axis=1)
        cols = slice(i * cfg.VW, (i + 1) * cfg.VW)
        in_maps.append({
            "xT": xTb,
            "wcat": _bf16(wcat),
            "bqk": np.ascontiguousarray(
                np.asarray(b_qkv, np.float32)[qk_rows]),
            "vbbc": np.ascontiguousarray(np.tile(
                np.asarray(b_qkv, np.float32)[v_rows][None, :], (128, 1))),
            "wdT": _bf16(np.asarray(w_dense, np.float32)[:, cols].T),
            "cosT": cos128,
            "sinT": sin128s,
            "maskT": maskT,
        })
    return in_maps


def combine_outputs(cfg: Cfg, results, b_dense):
    acc = np.zeros((cfg.E, cfg.SF), dtype=np.float64)
    for r in results:
        acc += np.asarray(r["outT"]).astype(np.float64)
    out = acc.T.reshape(cfg.B, cfg.S, cfg.E) + \
        np.asarray(b_dense, np.float64)
    return out.astype(np.float32)


_PROGRAM_CACHE = {}


def kernel(x, w_qkv, b_qkv, w_dense, b_dense):
    from concourse.bass_utils import run_bass_kernel_spmd

    cfg = Cfg()
    key = "full"
    if key not in _PROGRAM_CACHE:
        _PROGRAM_CACHE[key] = build_program(cfg)
    nc = _PROGRAM_CACHE[key]
    in_maps = make_in_maps(cfg, np.asarray(x), np.asarray(w_qkv),
                           np.asarray(b_qkv), np.asarray(w_dense))
    res = run_bass_kernel_spmd(nc, in_maps, list(range(cfg.n_cores)))
    return combine_outputs(cfg, res.results, np.asarray(b_dense))


# revision 50
# speedup vs baseline: 1.1343x; 1.0273x over previous
"""GPT-NeoX attention layer (B=2, S=2048, E=2048, H=16, partial RoPE 32/128)
as a Bass/Tile kernel for 8 Trainium2 NeuronCores.

Sharding: tensor-parallel across heads (2 heads per core, Megatron-style).
Each core computes QKV projection for its 2 heads over all tokens, applies
partial RoPE, runs causal attention, and produces a partial dense output
(contraction over its 256 columns of w_dense).  The 8 bf16 partial outputs
are summed on the host and the dense bias is added once on the host.

Everything on-device is bf16 (inputs pre-converted on the host); PSUM
accumulation stays fp32.  Key structure choices:

  qk_sb  [128, 4, SF]   Q^T/K^T per head (head dim on partitions) - scores
                        and y^T matmuls consume this directly.
  vnat   [128, SF/128, 256]  V in NATURAL [token, d] layout, produced in
                        phase 1 by x-stationary matmuls (x as lhsT), so no
                        PE transposes of V are ever needed.
  scores S^T = K^T.T @ Q^T in [sk, sq] blocks; exp on ScalarE (pipelined one
                        block behind the scores matmuls).
  softmax sums          via N=1 matmuls with the exp'd block as the
                        stationary operand (out [sq,1]): nearly free on PE,
                        instead of a 512-wide ones-matmul per block.
  normalize             reciprocal -> tiny PE transpose -> GPSIMD
                        partition_broadcast -> one DVE multiply per chunk.
  dense                 interleaved into later attention heads (fills the
                        tensor engine while ScalarE works through exp).
"""

import numpy as np
from contextlib import ExitStack

import concourse.bass as bass
import concourse.bacc as bacc
import concourse.mybir as mybir
import concourse.tile as tile
from concourse.masks import make_identity

AF = mybir.ActivationFunctionType
F32 = mybir.dt.float32
BF16 = mybir.dt.bfloat16

NEG_MASK = -1.0e9


class Cfg:
    def __init__(self, B=2, S=2048, E=2048, H=16, n_cores=8):
        self.B, self.S, self.E, self.H = B, S, E, H
        self.HS = 128                  # head size (one partition tile)
        self.ROT = 32                  # rotary dims
        self.n_cores = n_cores
        self.HPC = H // n_cores        # heads per core
        assert self.HPC == 2, "kernel assumes 2 heads per core"
        self.NQK = 2 * self.HPC        # q/k row tiles (h0q,h0k,h1q,h1k)
        self.VW = self.HPC * self.HS   # v natural width (d per core)
        self.RW = self.NQK * self.HS   # per-core q+k rows
        self.WCOLS = self.RW + self.VW
        self.SF = B * S
        self.KT = E // 128             # contraction tiles
        self.SC = 256                  # phase-1 token chunk
        self.NP1 = self.SF // self.SC
        self.G = self.SF // 4          # rope regroup width
        self.NQC = S // 512            # q chunks per (b, h)
        self.EO = E // 128             # dense output row tiles
        self.CT = self.HPC             # dense contraction tiles
        self.SCALE = 1.0 / np.sqrt(self.HS)
        assert S % 512 == 0 and E % 128 == 0 and self.SF % (4 * self.SC) == 0


class _Feeder:
    """FIFO of deferred dense micro-step generators, materialized from
    (b, scp) specs once `factory` is set and the feeder is enabled."""

    def __init__(self):
        self.specs = []
        self.gens = []
        self.factory = None
        self.enabled = False

    def push(self, spec):
        self.specs.append(spec)

    def _refill(self):
        if not self.gens and self.specs and self.factory:
            self.gens.append(self.factory(*self.specs.pop(0)))

    def step(self):
        if not self.enabled:
            return
        self._refill()
        while self.gens:
            try:
                next(self.gens[0])
                return
            except StopIteration:
                self.gens.pop(0)
                self._refill()

    def drain(self):
        assert self.enabled
        while True:
            self._refill()
            if not self.gens:
                return
            for _ in self.gens.pop(0):
                pass


class _Pump:
    """Steps an attention generator one j-block at a time; the generator
    yields an int (p1 chunks that must be emitted first) before each chunk
    and None per block."""

    def __init__(self, gen):
        self.gen = gen
        self.parked = None
        self.done = False

    def step(self, sc):
        if self.done:
            return False
        if self.parked is not None:
            if self.parked > sc:
                return False
            self.parked = None
        while True:
            try:
                v = next(self.gen)
            except StopIteration:
                self.done = True
                return False
            if v is None:
                return True
            if v > sc:
                self.parked = v
                return False


def build_program(cfg: Cfg, debug: bool = False) -> bass.Bass:
    B, S, E = cfg.B, cfg.S, cfg.E
    SF, KT, G = cfg.SF, cfg.KT, cfg.G
    SC, NQK, VW, RW = cfg.SC, cfg.NQK, cfg.VW, cfg.RW
    HPC, CT, EO = cfg.HPC, cfg.CT, cfg.EO
    NT = SF // 128                   # vnat token tiles

    nc = bacc.Bacc(None)
    xT = nc.dram_tensor("xT", [E, SF], BF16, kind="ExternalInput")
    wcat = nc.dram_tensor("wcat", [E, cfg.WCOLS], BF16, kind="ExternalInput")
    bqk = nc.dram_tensor("bqk", [RW], F32, kind="ExternalInput")
    vbbc = nc.dram_tensor("vbbc", [128, VW], F32, kind="ExternalInput")
    wdT = nc.dram_tensor("wdT", [VW, E], BF16, kind="ExternalInput")
    cosT = nc.dram_tensor("cosT", [32, SF], BF16, kind="ExternalInput")
    sinT = nc.dram_tensor("sinT", [32, SF], BF16, kind="ExternalInput")
    maskT = nc.dram_tensor("maskT", [128, 128], BF16, kind="ExternalInput")
    outT = nc.dram_tensor("outT", [E, SF], BF16, kind="ExternalOutput")

    with tile.TileContext(nc) as tc, ExitStack() as stk:
        consts = stk.enter_context(tc.tile_pool(name="consts", bufs=1))
        bigp = stk.enter_context(tc.tile_pool(name="big", bufs=1))
        qk_sb = bigp.tile([128, NQK, SF], BF16)
        vnat = bigp.tile([128, NT, VW], BF16)
        yT_sb = bigp.tile([128, CT, SF], BF16)

        # constants (tiles declared here; filled during phase-1 emission,
        # after the critical w/x DMA stream is issued)
        ident = consts.tile([128, 128], F32)
        identB = consts.tile([128, 128], BF16)
        ones_col = consts.tile([128, 1], BF16)
        mask_sb = consts.tile([128, 128], BF16)
        bqk_sb = consts.tile([128, NQK], F32)
        vb_sb = consts.tile([128, VW], F32)
        cos_sb = consts.tile([32, SF], BF16, tag="costab")
        sin_sb = consts.tile([32, SF], BF16, tag="sintab")
        wd_sb = consts.tile([128, CT, E], BF16, tag="wd")

        # RoPE: rotate_half is a partition swap within the 32 rot rows ->
        # DVE stream_shuffle (per-quadrant permutation) + elementwise
        # combine in [32, cols] layout, zero DMAs.  Each 1024-col slice is
        # emitted as soon as the phase-1 chunks covering it are done.
        SW = 1024
        rope_mask = [(i + 16) % 32 for i in range(32)]
        ropep = stk.enter_context(tc.tile_pool(name="rope", bufs=2))

        def rope_slice(i, sl):
            cs = slice(sl * SW, (sl + 1) * SW)
            blk = qk_sb[0:cfg.ROT, i, cs]
            sw = ropep.tile([32, SW], BF16, tag="swap", name="sw")
            nc.vector.stream_shuffle(sw, blk, rope_mask)
            with nc.allow_low_precision(reason="bf16 rope"):
                nc.vector.tensor_mul(sw, sw, sin_sb[:, cs])
                nc.vector.tensor_mul(blk, blk, cos_sb[:, cs])
                nc.vector.tensor_add(blk, blk, sw)

        # ---------------- Attention pools (live through phase 1) ----------
        feeder = _Feeder()
        ptp = stk.enter_context(tc.tile_pool(name="pstrip", bufs=6))
        npool = stk.enter_context(tc.tile_pool(name="norm", bufs=2))
        psA = stk.enter_context(tc.tile_pool(name="psA", bufs=2, space="PSUM"))
        psY = stk.enter_context(tc.tile_pool(name="psY", bufs=2, space="PSUM"))
        psS = stk.enter_context(tc.tile_pool(name="psS", bufs=1, space="PSUM"))
        LAG = 3   # j-blocks between scores+exp emission and yacc+sums

        def attention(b, hl, on_chain=None):
            """Generator: yields the p1-chunk prerequisite (int) before each
            q-chunk, then None after each emitted j-block."""
            scol = b * S
            q_t = qk_sb[:, 2 * hl, scol:scol + S]
            k_t = qk_sb[:, 2 * hl + 1, scol:scol + S]

            def emit_chain(c, psYt, psSt):
                recip = npool.tile([128, 4], BF16, tag="recip")
                with nc.allow_low_precision(reason="bf16 recip"):
                    nc.vector.reciprocal(recip, psSt[:, 0:4])
                # transpose each recip column to partition 0 ([1, 128])
                rps = psA.tile([128, 512], BF16, tag="A", name="rps")
                for g in range(4):
                    nc.tensor.matmul(
                        rps[0:1, 128 * g:128 * (g + 1)],
                        recip[:, g:g + 1], identB,
                        is_transpose=True, start=(g == 0), stop=(g == 3),
                        skip_group_check=True)
                rT = npool.tile([1, 512], F32, tag="rT")
                nc.vector.tensor_copy(rT, rps[0:1, 0:512])
                bc = npool.tile([128, 512], F32, tag="bc")
                for g in range(4):
                    nc.gpsimd.partition_broadcast(
                        bc[:, 128 * g:128 * (g + 1)],
                        rT[0:1, 128 * g:128 * (g + 1)])
                with nc.allow_low_precision(reason="bf16 y eviction"):
                    nc.vector.tensor_mul(
                        yT_sb[:, hl, scol + c * 512:scol + (c + 1) * 512],
                        psYt[:, 0:512], bc)
                if on_chain is not None:
                    on_chain(c)

            for c in range(cfg.NQC):
                # p1 chunks needed: data cols + one extra chunk so the rope
                # DVE work emitted at the slice boundary is already done
                sl_needed = (b * S + 512 * (c + 1) - 1) // SW
                yield (sl_needed + 1) * (SW // SC) + hl
                nj = 4 * (c + 1)
                psYt = psY.tile([128, 512], F32, tag="Y")
                psSt = psS.tile([128, 4], F32, tag="S")
                pend = []

                def emit_ys(j, pT, off, g0, psYt=psYt, psSt=psSt, c=c,
                            nj=nj):
                    nc.tensor.matmul(
                        psYt[:, off:512],
                        vnat[:, b * (S // 128) + j, 128 * hl:128 * (hl + 1)],
                        pT[:, off:512],
                        start=(j == 0), stop=(j == nj - 1),
                        skip_group_check=True)
                    for g in range(g0, 4):
                        # start only on the very first sums matmul of the
                        # chunk (bank-wide zero region); later columns
                        # accumulate onto pending-zero bytes.
                        nc.tensor.matmul(
                            psSt[:, g:g + 1],
                            pT[:, 128 * g:128 * (g + 1)], ones_col,
                            start=(j == 0 and g == 0),
                            stop=(j == nj - 1 and g == 3),
                            skip_group_check=True)

                for j in range(nj):
                    g0 = max(0, j - 4 * c)
                    off = 128 * g0
                    ps = psA.tile([128, 512], F32, tag="A")
                    nc.tensor.matmul(
                        ps[:, off:512],
                        k_t[:, 128 * j:128 * (j + 1)],
                        q_t[:, c * 512 + off:(c + 1) * 512],
                        start=True, stop=True, skip_group_check=True)
                    pT = ptp.tile([128, 512], BF16, tag=f"p{hl}", name="pT")
                    nc.scalar.activation(
                        pT[:, off:512], ps[:, off:512], AF.Exp,
                        scale=cfg.SCALE)
                    if j >= 4 * c:
                        # causal mask as a cheap post-exp 0/1 multiply
                        with nc.allow_low_precision(reason="bf16 mask"):
                            nc.vector.tensor_mul(
                                pT[:, off:off + 128],
                                pT[:, off:off + 128], mask_sb)
                    if len(pend) >= LAG:
                        emit_ys(*pend.pop(0))
                    feeder.step()
                    feeder.step()
                    pend.append((j, pT, off, g0))
                    yield None
                while pend:
                    emit_ys(*pend.pop(0))
                feeder.step()
                # chunk-end chain emitted atomically (the single psS bank
                # must be read here before another head's sums start)
                emit_chain(c, psYt, psSt)

        def make_pump(b, hl):
            hook = None
            if hl == HPC - 1:
                def hook(c, b=b):
                    if c % 2 == 1:
                        feeder.push((b, (c - 1) // 2))
            return _Pump(attention(b, hl, on_chain=hook))

        pumps_b0 = [make_pump(0, hl) for hl in range(HPC)]
        slot_budget = [0]
        active = [0]

        def inject(sc):
            # Called at points inside phase-1 emission; injects one ready
            # attention j-block into the PE stream.  Sticky generator choice:
            # a generator only loses its turn at a chunk boundary, so pool
            # tiles shared across generators (psS, psY) stay chunk-atomic.
            if slot_budget[0] <= 0:
                return
            n = len(pumps_b0)
            for off in range(n):
                p = pumps_b0[(active[0] + off) % n]
                if p.step(sc):
                    active[0] = (active[0] + off) % n
                    slot_budget[0] -= 1
                    return

        # ---------------- Phase 1: QKV projection ------------------------
        with tc.tile_pool(name="wq", bufs=1) as wp, \
             tc.tile_pool(name="xs", bufs=2) as xp, \
             tc.tile_pool(name="ps1", bufs=1, space="PSUM") as pp1:
            w_sb = wp.tile([128, KT, cfg.WCOLS], BF16)
            w_view = wcat.rearrange("(kt p) r -> p kt r", p=128)
            x_view = xT.rearrange("(kt p) s -> p kt s", p=128)

            # interleave per-kt w loads with quarters of the first x chunk
            xt0 = xp.tile([128, KT, SC], BF16, tag="xt")
            ktg = max(1, KT // 4)
            for q0 in range(0, KT, ktg):
                q1 = min(q0 + ktg, KT)
                for kt in range(q0, q1):
                    nc.sync.dma_start(out=w_sb[:, kt, :], in_=w_view[:, kt, :])
                nc.sync.dma_start(out=xt0[:, q0:q1, :],
                                  in_=x_view[:, q0:q1, 0:SC])
            xt1 = xp.tile([128, KT, SC], BF16, tag="xt")
            nc.sync.dma_start(out=xt1[:, :, :], in_=x_view[:, :, SC:2 * SC])

            # constants (after the critical w/x stream)
            make_identity(nc, ident)
            with nc.allow_low_precision(reason="bf16 identity"):
                nc.vector.tensor_copy(identB, ident)
            nc.vector.memset(ones_col, 1.0)
            nc.sync.dma_start(out=mask_sb, in_=maskT[:, :])
            nc.sync.dma_start(out=bqk_sb,
                              in_=bqk.rearrange("(rt p) -> p rt", p=128))
            nc.sync.dma_start(out=vb_sb, in_=vbbc[:, :])
            nc.sync.dma_start(out=cos_sb, in_=cosT[:, :])
            nc.sync.dma_start(out=sin_sb, in_=sinT[:, :])
            nc.sync.dma_start(
                out=wd_sb[:, :, :],
                in_=wdT.rearrange("(ct p) e -> p ct e", p=128))

            ntile = SC // 128   # v token sub-tiles per chunk (=2)
            # group-major chunks: 6 sequential accumulation groups cycling
            # 3 single-buffered banks (paired groups share a bank via the
            # pending-zero trick), evicted inline as each group finishes.
            for sc in range(cfg.NP1):
                slot_budget[0] = 7
                if sc == 0:
                    xt = xt0
                elif sc == 1:
                    xt = xt1
                else:
                    xt = xp.tile([128, KT, SC], BF16, tag="xt")
                    nc.sync.dma_start(
                        out=xt[:, :, :],
                        in_=x_view[:, :, sc * SC:(sc + 1) * SC])
                def p1_mm(pt, grp, half, kt):
                    reg = pt[:, 256 * half:256 * half + 256]
                    fl, ll = (kt == 0), (kt == KT - 1)
                    if grp < NQK:
                        nc.tensor.matmul(
                            reg,
                            w_sb[:, kt, 128 * grp:128 * (grp + 1)],
                            xt[:, kt, :],
                            start=fl and half == 0, stop=ll and half == 1,
                            skip_group_check=True)
                    else:
                        t = grp - NQK
                        nc.tensor.matmul(
                            reg,
                            xt[:, kt, 128 * t:128 * (t + 1)],
                            w_sb[:, kt, RW:RW + VW],
                            start=fl and half == 0, stop=ll and half == 1,
                            skip_group_check=True)

                def p1_evict(pt, grp, half):
                    reg = pt[:, 256 * half:256 * half + 256]
                    if grp < NQK:
                        nc.scalar.activation(
                            qk_sb[:, grp, sc * SC:(sc + 1) * SC], reg,
                            AF.Identity, bias=bqk_sb[:, grp:grp + 1])
                    else:
                        t = grp - NQK
                        with nc.allow_low_precision(reason="bf16 v"):
                            nc.vector.tensor_add(
                                vnat[:, sc * ntile + t, :], reg, vb_sb)

                if sc < 2:
                    # kt-outer while the w stream is still arriving: consume
                    # each w[kt] across all 6 groups as soon as it lands
                    pts = [pp1.tile([128, 512], F32, tag=f"p1{p}",
                                    name=f"p1ps{p}") for p in range(3)]
                    for kt in range(KT):
                        for grp in range(6):
                            p1_mm(pts[grp // 2], grp, grp % 2, kt)
                    for grp in range(6):
                        p1_evict(pts[grp // 2], grp, grp % 2)
                else:
                    # group-major: 6 sequential groups over 3 single-buffered
                    # banks, each evicted inline as it finishes
                    for pair in range(3):
                        pt = pp1.tile([128, 512], F32, tag=f"p1{pair}",
                                      name=f"p1ps{pair}")
                        for half in range(2):
                            for kt in range(KT):
                                p1_mm(pt, 2 * pair + half, half, kt)
                                if kt % 4 == 3:
                                    inject(sc - 1)
                            p1_evict(pt, 2 * pair + half, half)
                if (sc + 1) % (SW // SC) == 0:
                    for i in range(NQK):
                        rope_slice(i, sc // (SW // SC))

        # ---------------- Tail: remaining attention + dense ----------------
        with tc.tile_pool(name="outsb", bufs=4) as op, \
             tc.tile_pool(name="psD", bufs=3, space="PSUM") as psD:

            def dense_steps(b, scp):
                # one (eo) output row-tile over two 512-token col chunks
                for eo in range(EO):
                    ot = op.tile([128, 1024], BF16, tag="out")
                    for t in range(2):
                        col = b * S + (2 * scp + t) * 512
                        pd = psD.tile([128, 512], F32, tag="D")
                        for ct in range(CT):
                            nc.tensor.matmul(
                                pd,
                                wd_sb[:, ct, 128 * eo:128 * (eo + 1)],
                                yT_sb[:, ct, col:col + 512],
                                start=(ct == 0), stop=(ct == CT - 1),
                                skip_group_check=True)
                        yield
                        with nc.allow_low_precision(reason="bf16 out"):
                            if (eo + t) % 2 == 0:
                                nc.vector.tensor_copy(
                                    ot[:, 512 * t:512 * (t + 1)], pd)
                            else:
                                nc.scalar.activation(
                                    ot[:, 512 * t:512 * (t + 1)], pd, AF.Copy)
                    nc.sync.dma_start(
                        out=outT[128 * eo:128 * (eo + 1),
                                 b * S + scp * 1024:b * S + (scp + 1) * 1024],
                        in_=ot)
                    yield

            feeder.factory = dense_steps
            feeder.enabled = True
            BIG = 10 ** 9
            while any(p.step(BIG) for p in pumps_b0):
                pass
            for bb in range(1, B):
                pumps = [make_pump(bb, hl) for hl in range(HPC)]
                for p in pumps:
                    while p.step(BIG):
                        pass
            feeder.drain()

        if debug:
            dqk = nc.dram_tensor("dbg_qk", [128, NQK, SF], BF16,
                                 kind="ExternalOutput")
            dv = nc.dram_tensor("dbg_v", [128, NT, VW], BF16,
                                kind="ExternalOutput")
            dy = nc.dram_tensor("dbg_y", [128, CT, SF], BF16,
                                kind="ExternalOutput")
            nc.sync.dma_start(out=dqk[:, :, :], in_=qk_sb[:, :, :])
            nc.sync.dma_start(out=dv[:, :, :], in_=vnat[:, :, :])
            nc.sync.dma_start(out=dy[:, :, :], in_=yT_sb[:, :, :])

    nc.finalize()
    return nc


# ---------------------------------------------------------------------------
# Host-side input preparation / sharding
# ---------------------------------------------------------------------------

def _bf16(a):
    import ml_dtypes
    return np.ascontiguousarray(a, np.float32).astype(ml_dtypes.bfloat16)


def _rope_tables(cfg: Cfg):
    inv_freq = 1.0 / (10000.0 ** (np.arange(0, cfg.ROT, 2, dtype=np.float64)
                                  / cfg.ROT))
    t = np.arange(cfg.S, dtype=np.float64)
    freqs = np.outer(t, inv_freq)                       # [S, 16]
    emb = np.concatenate([freqs, freqs], axis=-1)       # [S, 32]
    cos = np.cos(emb).T.astype(np.float32)              # [32, S]
    sin = np.sin(emb).T.astype(np.float32)
    cosF = np.tile(cos, (1, cfg.B))                     # [32, SF]
    sinF = np.tile(sin, (1, cfg.B))
    sinF[:cfg.ROT // 2] *= -1.0                         # fold rotate_half sign
    return _bf16(cosF), _bf16(sinF)


def make_in_maps(cfg: Cfg, x, w_qkv, b_qkv, w_dense):
    HS, HPC = cfg.HS, cfg.HPC
    xTb = _bf16(np.ascontiguousarray(
        np.asarray(x, np.float32).reshape(cfg.SF, cfg.E).T))
    cos128, sin128s = _rope_tables(cfg)
    p = np.arange(128)[:, None]
    f = np.arange(128)[None, :]
    maskT = _bf16(np.where(p <= f, 1.0, 0.0))   # post-exp 0/1 causal mask
    in_maps = []
    for i in range(cfg.n_cores):
        heads = [HPC * i + h for h in range(HPC)]
        qk_rows = np.concatenate(
            [np.arange(h * 3 * HS + qk * HS, h * 3 * HS + (qk + 1) * HS)
             for h in heads for qk in range(2)])
        v_rows = np.concatenate(
            [np.arange(h * 3 * HS + 2 * HS, h * 3 * HS + 3 * HS)
             for h in heads])
        wcat = np.concatenate(
            [np.asarray(w_qkv, np.float32)[qk_rows, :].T,
             np.asarray(w_qkv, np.float32)[v_rows, :].T], axis=1)
        cols = slice(i * cfg.VW, (i + 1) * cfg.VW)
        in_maps.append({
            "xT": xTb,
            "wcat": _bf16(wcat),
            "bqk": np.ascontiguousarray(
                np.asarray(b_qkv, np.float32)[qk_rows]),
            "vbbc": np.ascontiguousarray(np.tile(
                np.asarray(b_qkv, np.float32)[v_rows][None, :], (128, 1))),
            "wdT": _bf16(np.asarray(w_dense, np.float32)[:, cols].T),
            "cosT": cos128,
            "sinT": sin128s,
            "maskT": maskT,
        })
    return in_maps


def combine_outputs(cfg: Cfg, results, b_dense):
    acc = np.zeros((cfg.E, cfg.SF), dtype=np.float64)
    for r in results:
        acc += np.asarray(r["outT"]).astype(np.float64)
    out = acc.T.reshape(cfg.B, cfg.S, cfg.E) + \
        np.asarray(b_dense, np.float64)
    return out.astype(np.float32)


_PROGRAM_CACHE = {}


def kernel(x, w_qkv, b_qkv, w_dense, b_dense):
    from concourse.bass_utils import run_bass_kernel_spmd

    cfg = Cfg()
    key = "full"
    if key not in _PROGRAM_CACHE:
        _PROGRAM_CACHE[key] = build_program(cfg)
    nc = _PROGRAM_CACHE[key]
    in_maps = make_in_maps(cfg, np.asarray(x), np.asarray(w_qkv),
                           np.asarray(b_qkv), np.asarray(w_dense))
    res = run_bass_kernel_spmd(nc, in_maps, list(range(cfg.n_cores)))
    return combine_outputs(cfg, res.results, np.asarray(b_dense))


# revision 51
# speedup vs baseline: 1.1580x; 1.0209x over previous
"""GPT-NeoX attention layer (B=2, S=2048, E=2048, H=16, partial RoPE 32/128)
as a Bass/Tile kernel for 8 Trainium2 NeuronCores.

Sharding: tensor-parallel across heads (2 heads per core, Megatron-style).
Each core computes QKV projection for its 2 heads over all tokens, applies
partial RoPE, runs causal attention, and produces a partial dense output
(contraction over its 256 columns of w_dense).  The 8 bf16 partial outputs
are summed on the host and the dense bias is added once on the host.

Everything on-device is bf16 (inputs pre-converted on the host); PSUM
accumulation stays fp32.  Key structure choices:

  qk_sb  [128, 4, SF]   Q^T/K^T per head (head dim on partitions) - scores
                        and y^T matmuls consume this directly.
  vnat   [128, SF/128, 256]  V in NATURAL [token, d] layout, produced in
                        phase 1 by x-stationary matmuls (x as lhsT), so no
                        PE transposes of V are ever needed.
  scores S^T = K^T.T @ Q^T in [sk, sq] blocks; exp on ScalarE (pipelined one
                        block behind the scores matmuls).
  softmax sums          via N=1 matmuls with the exp'd block as the
                        stationary operand (out [sq,1]): nearly free on PE,
                        instead of a 512-wide ones-matmul per block.
  normalize             reciprocal -> tiny PE transpose -> GPSIMD
                        partition_broadcast -> one DVE multiply per chunk.
  dense                 interleaved into later attention heads (fills the
                        tensor engine while ScalarE works through exp).
"""

import numpy as np
from contextlib import ExitStack

import concourse.bass as bass
import concourse.bacc as bacc
import concourse.mybir as mybir
import concourse.tile as tile
from concourse.masks import make_identity

AF = mybir.ActivationFunctionType
F32 = mybir.dt.float32
BF16 = mybir.dt.bfloat16

NEG_MASK = -1.0e9


class Cfg:
    def __init__(self, B=2, S=2048, E=2048, H=16, n_cores=8):
        self.B, self.S, self.E, self.H = B, S, E, H
        self.HS = 128                  # head size (one partition tile)
        self.ROT = 32                  # rotary dims
        self.n_cores = n_cores
        self.HPC = H // n_cores        # heads per core
        assert self.HPC == 2, "kernel assumes 2 heads per core"
        self.NQK = 2 * self.HPC        # q/k row tiles (h0q,h0k,h1q,h1k)
        self.VW = self.HPC * self.HS   # v natural width (d per core)
        self.RW = self.NQK * self.HS   # per-core q+k rows
        self.WCOLS = self.RW + self.VW
        self.SF = B * S
        self.KT = E // 128             # contraction tiles
        self.SC = 256                  # phase-1 token chunk
        self.NP1 = self.SF // self.SC
        self.G = self.SF // 4          # rope regroup width
        self.NQC = S // 512            # q chunks per (b, h)
        self.EO = E // 128             # dense output row tiles
        self.CT = self.HPC             # dense contraction tiles
        self.SCALE = 1.0 / np.sqrt(self.HS)
        assert S % 512 == 0 and E % 128 == 0 and self.SF % (4 * self.SC) == 0


class _Feeder:
    """FIFO of deferred dense micro-step generators, materialized from
    (b, scp) specs once `factory` is set and the feeder is enabled."""

    def __init__(self):
        self.specs = []
        self.gens = []
        self.factory = None
        self.enabled = False

    def push(self, spec):
        self.specs.append(spec)

    def _refill(self):
        if not self.gens and self.specs and self.factory:
            self.gens.append(self.factory(*self.specs.pop(0)))

    def step(self):
        if not self.enabled:
            return
        self._refill()
        while self.gens:
            try:
                next(self.gens[0])
                return
            except StopIteration:
                self.gens.pop(0)
                self._refill()

    def drain(self):
        assert self.enabled
        while True:
            self._refill()
            if not self.gens:
                return
            for _ in self.gens.pop(0):
                pass


class _Pump:
    """Steps an attention generator one j-block at a time; the generator
    yields an int (p1 chunks that must be emitted first) before each chunk
    and None per block."""

    def __init__(self, gen):
        self.gen = gen
        self.parked = None
        self.done = False

    def step(self, sc):
        if self.done:
            return False
        if self.parked is not None:
            if self.parked > sc:
                return False
            self.parked = None
        while True:
            try:
                v = next(self.gen)
            except StopIteration:
                self.done = True
                return False
            if v is None:
                return True
            if v > sc:
                self.parked = v
                return False


def build_program(cfg: Cfg, debug: bool = False) -> bass.Bass:
    B, S, E = cfg.B, cfg.S, cfg.E
    SF, KT, G = cfg.SF, cfg.KT, cfg.G
    SC, NQK, VW, RW = cfg.SC, cfg.NQK, cfg.VW, cfg.RW
    HPC, CT, EO = cfg.HPC, cfg.CT, cfg.EO
    NT = SF // 128                   # vnat token tiles

    nc = bacc.Bacc(None)
    xT = nc.dram_tensor("xT", [E, SF], BF16, kind="ExternalInput")
    wcat = nc.dram_tensor("wcat", [E, cfg.WCOLS], BF16, kind="ExternalInput")
    bqk = nc.dram_tensor("bqk", [RW], F32, kind="ExternalInput")
    vbbc = nc.dram_tensor("vbbc", [128, VW], F32, kind="ExternalInput")
    wdT = nc.dram_tensor("wdT", [VW, E], BF16, kind="ExternalInput")
    cosT = nc.dram_tensor("cosT", [32, SF], BF16, kind="ExternalInput")
    sinT = nc.dram_tensor("sinT", [32, SF], BF16, kind="ExternalInput")
    maskT = nc.dram_tensor("maskT", [128, 128], BF16, kind="ExternalInput")
    outT = nc.dram_tensor("outT", [E, SF], BF16, kind="ExternalOutput")

    with tile.TileContext(nc) as tc, ExitStack() as stk:
        consts = stk.enter_context(tc.tile_pool(name="consts", bufs=1))
        bigp = stk.enter_context(tc.tile_pool(name="big", bufs=1))
        qk_sb = bigp.tile([128, NQK, SF], BF16)
        vnat = bigp.tile([128, NT, VW], BF16)
        yT_sb = bigp.tile([128, CT, SF], BF16)

        # constants (tiles declared here; filled during phase-1 emission,
        # after the critical w/x DMA stream is issued)
        ident = consts.tile([128, 128], F32)
        identB = consts.tile([128, 128], BF16)
        ones_col = consts.tile([128, 1], BF16)
        mask_sb = consts.tile([128, 128], BF16)
        bqk_sb = consts.tile([128, NQK], F32)
        vb_sb = consts.tile([128, VW], F32)
        cos_sb = consts.tile([32, SF], BF16, tag="costab")
        sin_sb = consts.tile([32, SF], BF16, tag="sintab")
        wd_sb = consts.tile([128, CT, E], BF16, tag="wd")

        # RoPE: rotate_half is a partition swap within the 32 rot rows ->
        # DVE stream_shuffle (per-quadrant permutation) + elementwise
        # combine in [32, cols] layout, zero DMAs.  Each 1024-col slice is
        # emitted as soon as the phase-1 chunks covering it are done.
        SW = 1024
        rope_mask = [(i + 16) % 32 for i in range(32)]
        ropep = stk.enter_context(tc.tile_pool(name="rope", bufs=2))

        def rope_slice(i, sl):
            cs = slice(sl * SW, (sl + 1) * SW)
            blk = qk_sb[0:cfg.ROT, i, cs]
            sw = ropep.tile([32, SW], BF16, tag="swap", name="sw")
            nc.vector.stream_shuffle(sw, blk, rope_mask)
            with nc.allow_low_precision(reason="bf16 rope"):
                nc.vector.tensor_mul(sw, sw, sin_sb[:, cs])
                nc.vector.tensor_mul(blk, blk, cos_sb[:, cs])
                nc.vector.tensor_add(blk, blk, sw)

        # ---------------- Attention pools (live through phase 1) ----------
        feeder = _Feeder()
        ptp = stk.enter_context(tc.tile_pool(name="pstrip", bufs=6))
        npool = stk.enter_context(tc.tile_pool(name="norm", bufs=2))
        psA = stk.enter_context(tc.tile_pool(name="psA", bufs=2, space="PSUM"))
        psY = stk.enter_context(tc.tile_pool(name="psY", bufs=2, space="PSUM"))
        psS = stk.enter_context(tc.tile_pool(name="psS", bufs=1, space="PSUM"))
        LAG = 3   # j-blocks between scores+exp emission and yacc+sums

        def attention(b, hl, on_chain=None):
            """Generator: yields the p1-chunk prerequisite (int) before each
            q-chunk, then None after each emitted j-block."""
            scol = b * S
            q_t = qk_sb[:, 2 * hl, scol:scol + S]
            k_t = qk_sb[:, 2 * hl + 1, scol:scol + S]

            def emit_chain(c, psYt, psSt):
                recip = npool.tile([128, 4], BF16, tag="recip")
                with nc.allow_low_precision(reason="bf16 recip"):
                    nc.vector.reciprocal(recip, psSt[:, 0:4])
                # gather per-partition recips to partition 0 via one small
                # strided SBUF->SBUF DMA: rT3[0, s, g] = recip[s, g]
                rT3 = npool.tile([1, 128, 4], BF16, tag="rT")
                nc.sync.dma_start(out=rT3[0:1, :, :], in_=recip[:, 0:4])
                bc = npool.tile([128, 512], BF16, tag="bc")
                for g in range(4):
                    nc.gpsimd.partition_broadcast(
                        bc[:, 128 * g:128 * (g + 1)], rT3[0:1, :, g:g + 1])
                with nc.allow_low_precision(reason="bf16 y eviction"):
                    nc.vector.tensor_mul(
                        yT_sb[:, hl, scol + c * 512:scol + (c + 1) * 512],
                        psYt[:, 0:512], bc)
                if on_chain is not None:
                    on_chain(c)

            for c in range(cfg.NQC):
                # p1 chunks needed: data cols + one extra chunk so the rope
                # DVE work emitted at the slice boundary is already done
                sl_needed = (b * S + 512 * (c + 1) - 1) // SW
                yield (sl_needed + 1) * (SW // SC) + hl
                nj = 4 * (c + 1)
                psYt = psY.tile([128, 512], F32, tag="Y")
                psSt = psS.tile([128, 4], F32, tag="S")
                pend = []

                def emit_ys(j, pT, off, g0, psYt=psYt, psSt=psSt, c=c,
                            nj=nj):
                    nc.tensor.matmul(
                        psYt[:, off:512],
                        vnat[:, b * (S // 128) + j, 128 * hl:128 * (hl + 1)],
                        pT[:, off:512],
                        start=(j == 0), stop=(j == nj - 1),
                        skip_group_check=True)
                    for g in range(g0, 4):
                        # start only on the very first sums matmul of the
                        # chunk (bank-wide zero region); later columns
                        # accumulate onto pending-zero bytes.
                        nc.tensor.matmul(
                            psSt[:, g:g + 1],
                            pT[:, 128 * g:128 * (g + 1)], ones_col,
                            start=(j == 0 and g == 0),
                            stop=(j == nj - 1 and g == 3),
                            skip_group_check=True)

                for j in range(nj):
                    g0 = max(0, j - 4 * c)
                    off = 128 * g0
                    ps = psA.tile([128, 512], F32, tag="A")
                    nc.tensor.matmul(
                        ps[:, off:512],
                        k_t[:, 128 * j:128 * (j + 1)],
                        q_t[:, c * 512 + off:(c + 1) * 512],
                        start=True, stop=True, skip_group_check=True)
                    pT = ptp.tile([128, 512], BF16, tag=f"p{hl}", name="pT")
                    nc.scalar.activation(
                        pT[:, off:512], ps[:, off:512], AF.Exp,
                        scale=cfg.SCALE)
                    if j >= 4 * c:
                        # causal mask as a cheap post-exp 0/1 multiply
                        with nc.allow_low_precision(reason="bf16 mask"):
                            nc.vector.tensor_mul(
                                pT[:, off:off + 128],
                                pT[:, off:off + 128], mask_sb)
                    if len(pend) >= LAG:
                        emit_ys(*pend.pop(0))
                    feeder.step()
                    feeder.step()
                    pend.append((j, pT, off, g0))
                    yield None
                while pend:
                    emit_ys(*pend.pop(0))
                feeder.step()
                # chunk-end chain emitted atomically (the single psS bank
                # must be read here before another head's sums start)
                emit_chain(c, psYt, psSt)

        def make_pump(b, hl):
            hook = None
            if hl == HPC - 1:
                def hook(c, b=b):
                    if c % 2 == 1:
                        feeder.push((b, (c - 1) // 2))
            return _Pump(attention(b, hl, on_chain=hook))

        pumps_b0 = [make_pump(0, hl) for hl in range(HPC)]
        slot_budget = [0]
        active = [0]

        def inject(sc):
            # Called at points inside phase-1 emission; injects one ready
            # attention j-block into the PE stream.  Sticky generator choice:
            # a generator only loses its turn at a chunk boundary, so pool
            # tiles shared across generators (psS, psY) stay chunk-atomic.
            if slot_budget[0] <= 0:
                return
            n = len(pumps_b0)
            for off in range(n):
                p = pumps_b0[(active[0] + off) % n]
                if p.step(sc):
                    active[0] = (active[0] + off) % n
                    slot_budget[0] -= 1
                    return

        # ---------------- Phase 1: QKV projection ------------------------
        with tc.tile_pool(name="wq", bufs=1) as wp, \
             tc.tile_pool(name="xs", bufs=2) as xp, \
             tc.tile_pool(name="ps1", bufs=1, space="PSUM") as pp1:
            w_sb = wp.tile([128, KT, cfg.WCOLS], BF16)
            w_view = wcat.rearrange("(kt p) r -> p kt r", p=128)
            x_view = xT.rearrange("(kt p) s -> p kt s", p=128)

            # interleave per-kt w loads with quarters of the first x chunk
            xt0 = xp.tile([128, KT, SC], BF16, tag="xt")
            ktg = max(1, KT // 4)
            for q0 in range(0, KT, ktg):
                q1 = min(q0 + ktg, KT)
                for kt in range(q0, q1):
                    nc.sync.dma_start(out=w_sb[:, kt, :], in_=w_view[:, kt, :])
                nc.sync.dma_start(out=xt0[:, q0:q1, :],
                                  in_=x_view[:, q0:q1, 0:SC])
            xt1 = xp.tile([128, KT, SC], BF16, tag="xt")
            nc.sync.dma_start(out=xt1[:, :, :], in_=x_view[:, :, SC:2 * SC])

            # constants (after the critical w/x stream)
            make_identity(nc, ident)
            with nc.allow_low_precision(reason="bf16 identity"):
                nc.vector.tensor_copy(identB, ident)
            nc.vector.memset(ones_col, 1.0)
            nc.sync.dma_start(out=mask_sb, in_=maskT[:, :])
            nc.sync.dma_start(out=bqk_sb,
                              in_=bqk.rearrange("(rt p) -> p rt", p=128))
            nc.sync.dma_start(out=vb_sb, in_=vbbc[:, :])
            nc.sync.dma_start(out=cos_sb, in_=cosT[:, :])
            nc.sync.dma_start(out=sin_sb, in_=sinT[:, :])
            nc.sync.dma_start(
                out=wd_sb[:, :, :],
                in_=wdT.rearrange("(ct p) e -> p ct e", p=128))

            ntile = SC // 128   # v token sub-tiles per chunk (=2)
            # group-major chunks: 6 sequential accumulation groups cycling
            # 3 single-buffered banks (paired groups share a bank via the
            # pending-zero trick), evicted inline as each group finishes.
            for sc in range(cfg.NP1):
                slot_budget[0] = 7
                if sc == 0:
                    xt = xt0
                elif sc == 1:
                    xt = xt1
                else:
                    xt = xp.tile([128, KT, SC], BF16, tag="xt")
                    nc.sync.dma_start(
                        out=xt[:, :, :],
                        in_=x_view[:, :, sc * SC:(sc + 1) * SC])
                def p1_mm(pt, grp, half, kt):
                    reg = pt[:, 256 * half:256 * half + 256]
                    fl, ll = (kt == 0), (kt == KT - 1)
                    if grp < NQK:
                        nc.tensor.matmul(
                            reg,
                            w_sb[:, kt, 128 * grp:128 * (grp + 1)],
                            xt[:, kt, :],
                            start=fl and half == 0, stop=ll and half == 1,
                            skip_group_check=True)
                    else:
                        t = grp - NQK
                        nc.tensor.matmul(
                            reg,
                            xt[:, kt, 128 * t:128 * (t + 1)],
                            w_sb[:, kt, RW:RW + VW],
                            start=fl and half == 0, stop=ll and half == 1,
                            skip_group_check=True)

                def p1_evict(pt, grp, half):
                    reg = pt[:, 256 * half:256 * half + 256]
                    if grp < NQK:
                        nc.scalar.activation(
                            qk_sb[:, grp, sc * SC:(sc + 1) * SC], reg,
                            AF.Identity, bias=bqk_sb[:, grp:grp + 1])
                    else:
                        t = grp - NQK
                        with nc.allow_low_precision(reason="bf16 v"):
                            nc.vector.tensor_add(
                                vnat[:, sc * ntile + t, :], reg, vb_sb)

                if sc < 2:
                    # kt-outer while the w stream is still arriving: consume
                    # each w[kt] across all 6 groups as soon as it lands
                    pts = [pp1.tile([128, 512], F32, tag=f"p1{p}",
                                    name=f"p1ps{p}") for p in range(3)]
                    for kt in range(KT):
                        for grp in range(6):
                            p1_mm(pts[grp // 2], grp, grp % 2, kt)
                    for grp in range(6):
                        p1_evict(pts[grp // 2], grp, grp % 2)
                else:
                    # group-major: 6 sequential groups over 3 single-buffered
                    # banks, each evicted inline as it finishes
                    for pair in range(3):
                        pt = pp1.tile([128, 512], F32, tag=f"p1{pair}",
                                      name=f"p1ps{pair}")
                        for half in range(2):
                            for kt in range(KT):
                                p1_mm(pt, 2 * pair + half, half, kt)
                                if kt % 4 == 3:
                                    inject(sc - 1)
                            p1_evict(pt, 2 * pair + half, half)
                if (sc + 1) % (SW // SC) == 0:
                    for i in range(NQK):
                        rope_slice(i, sc // (SW // SC))

        # ---------------- Tail: remaining attention + dense ----------------
        with tc.tile_pool(name="outsb", bufs=4) as op, \
             tc.tile_pool(name="psD", bufs=3, space="PSUM") as psD:

            def dense_steps(b, scp):
                # one (eo) output row-tile over two 512-token col chunks
                for eo in range(EO):
                    ot = op.tile([128, 1024], BF16, tag="out")
                    for t in range(2):
                        col = b * S + (2 * scp + t) * 512
                        pd = psD.tile([128, 512], F32, tag="D")
                        for ct in range(CT):
                            nc.tensor.matmul(
                                pd,
                                wd_sb[:, ct, 128 * eo:128 * (eo + 1)],
                                yT_sb[:, ct, col:col + 512],
                                start=(ct == 0), stop=(ct == CT - 1),
                                skip_group_check=True)
                        yield
                        with nc.allow_low_precision(reason="bf16 out"):
                            if (eo + t) % 2 == 0:
                                nc.vector.tensor_copy(
                                    ot[:, 512 * t:512 * (t + 1)], pd)
                            else:
                                nc.scalar.activation(
                                    ot[:, 512 * t:512 * (t + 1)], pd, AF.Copy)
                    nc.sync.dma_start(
                        out=outT[128 * eo:128 * (eo + 1),
                                 b * S + scp * 1024:b * S + (scp + 1) * 1024],
                        in_=ot)
                    yield

            feeder.factory = dense_steps
            feeder.enabled = True
            BIG = 10 ** 9
            while any(p.step(BIG) for p in pumps_b0):
                pass
            for bb in range(1, B):
                pumps = [make_pump(bb, hl) for hl in range(HPC)]
                for p in pumps:
                    while p.step(BIG):
                        pass
            feeder.drain()

        if debug:
            dqk = nc.dram_tensor("dbg_qk", [128, NQK, SF], BF16,
                                 kind="ExternalOutput")
            dv = nc.dram_tensor("dbg_v", [128, NT, VW], BF16,
                                kind="ExternalOutput")
            dy = nc.dram_tensor("dbg_y", [128, CT, SF], BF16,
                                kind="ExternalOutput")
            nc.sync.dma_start(out=dqk[:, :, :], in_=qk_sb[:, :, :])
            nc.sync.dma_start(out=dv[:, :, :], in_=vnat[:, :, :])
            nc.sync.dma_start(out=dy[:, :, :], in_=yT_sb[:, :, :])

    nc.finalize()
    return nc


# ---------------------------------------------------------------------------
# Host-side input preparation / sharding
# ---------------------------------------------------------------------------

def _bf16(a):
    import ml_dtypes
    return np.ascontiguousarray(a, np.float32).astype(ml_dtypes.bfloat16)


def _rope_tables(cfg: Cfg):
    inv_freq = 1.0 / (10000.0 ** (np.arange(0, cfg.ROT, 2, dtype=np.float64)
                                  / cfg.ROT))
    t = np.arange(cfg.S, dtype=np.float64)
    freqs = np.outer(t, inv_freq)                       # [S, 16]
    emb = np.concatenate([freqs, freqs], axis=-1)       # [S, 32]
    cos = np.cos(emb).T.astype(np.float32)              # [32, S]
    sin = np.sin(emb).T.astype(np.float32)
    cosF = np.tile(cos, (1, cfg.B))                     # [32, SF]
    sinF = np.tile(sin, (1, cfg.B))
    sinF[:cfg.ROT // 2] *= -1.0                         # fold rotate_half sign
    return _bf16(cosF), _bf16(sinF)


def make_in_maps(cfg: Cfg, x, w_qkv, b_qkv, w_dense):
    HS, HPC = cfg.HS, cfg.HPC
    xTb = _bf16(np.ascontiguousarray(
        np.asarray(x, np.float32).reshape(cfg.SF, cfg.E).T))
    cos128, sin128s = _rope_tables(cfg)
    p = np.arange(128)[:, None]
    f = np.arange(128)[None, :]
    maskT = _bf16(np.where(p <= f, 1.0, 0.0))   # post-exp 0/1 causal mask
    in_maps = []
    for i in range(cfg.n_cores):
        heads = [HPC * i + h for h in range(HPC)]
        qk_rows = np.concatenate(
            [np.arange(h * 3 * HS + qk * HS, h * 3 * HS + (qk + 1) * HS)
             for h in heads for qk in range(2)])
        v_rows = np.concatenate(
            [np.arange(h * 3 * HS + 2 * HS, h * 3 * HS + 3 * HS)
             for h in heads])
        wcat = np.concatenate(
            [np.asarray(w_qkv, np.float32)[qk_rows, :].T,
             np.asarray(w_qkv, np.float32)[v_rows, :].T], axis=1)
        cols = slice(i * cfg.VW, (i + 1) * cfg.VW)
        in_maps.append({
            "xT": xTb,
            "wcat": _bf16(wcat),
            "bqk": np.ascontiguousarray(
                np.asarray(b_qkv, np.float32)[qk_rows]),
            "vbbc": np.ascontiguousarray(np.tile(
                np.asarray(b_qkv, np.float32)[v_rows][None, :], (128, 1))),
            "wdT": _bf16(np.asarray(w_dense, np.float32)[:, cols].T),
            "cosT": cos128,
            "sinT": sin128s,
            "maskT": maskT,
        })
    return in_maps


def combine_outputs(cfg: Cfg, results, b_dense):
    acc = np.zeros((cfg.E, cfg.SF), dtype=np.float64)
    for r in results:
        acc += np.asarray(r["outT"]).astype(np.float64)
    out = acc.T.reshape(cfg.B, cfg.S, cfg.E) + \
        np.asarray(b_dense, np.float64)
    return out.astype(np.float32)


_PROGRAM_CACHE = {}


def kernel(x, w_qkv, b_qkv, w_dense, b_dense):
    from concourse.bass_utils import run_bass_kernel_spmd

    cfg = Cfg()
    key = "full"
    if key not in _PROGRAM_CACHE:
        _PROGRAM_CACHE[key] = build_program(cfg)
    nc = _PROGRAM_CACHE[key]
    in_maps = make_in_maps(cfg, np.asarray(x), np.asarray(w_qkv),
                           np.asarray(b_qkv), np.asarray(w_dense))
    res = run_bass_kernel_spmd(nc, in_maps, list(range(cfg.n_cores)))
    return combine_outputs(cfg, res.results, np.asarray(b_dense))
